# revision 1
# baseline (speedup 1.0000x reference)
"""Trainium2 Bass kernel for per-token multi-head self-attention.

Computation (per token t):
  q,k,v = x @ W{q,k,v}.T ; scores = (q_t k_t^T)/sqrt(128) over heads [16x16]
  out_t = softmax(scores) @ v_t ; y = out @ Wo.T

Sharding: data-parallel over the 16384 tokens -> 8 cores x 2048 tokens.
All activations flow on-chip in transposed ([feature, token]) layout; the
host pre-transposes x shards and weights so every matmul operand loads
naturally with the contraction dim on partitions (no on-chip transposes for
the 4 big matmuls). fp32r (full-rate tf32-like) for the big matmuls.

Middle stage per 4-token group: per-token 16x16 score matmuls -> exp (ACT)
into a block-diagonal [128,64] attn matrix -> one AV matmul against the
PE-transposed [4tok x 32, d] V block (with a ones column producing the
softmax normalizer Z) -> per-partition 1/Z scale -> PE-transpose back.
"""
import math
from contextlib import ExitStack

import numpy as np

NCORES = 8
E = 2048          # hidden
NH = 16           # heads
HD = 128          # head dim
TPC = 2048        # tokens per core
TC = 512          # token chunk in pass B
P = 128

_cached = {}


def _build_program():
    import concourse.bass as bass
    import concourse.tile as tile
    from concourse import bacc, mybir
    from concourse.masks import make_identity

    f32 = mybir.dt.float32
    f32r = mybir.dt.float32r

    nc = bacc.Bacc("TRN2", target_bir_lowering=False, debug=False)

    xT_d = nc.dram_tensor("xT", [E, TPC], f32r, kind="ExternalInput").ap()
    WqT_d = nc.dram_tensor("WqT", [E, E], f32r, kind="ExternalInput").ap()
    WkT_d = nc.dram_tensor("WkT", [E, E], f32r, kind="ExternalInput").ap()
    WvT_d = nc.dram_tensor("WvT", [E, E], f32r, kind="ExternalInput").ap()
    WoT_d = nc.dram_tensor("WoT", [E, E], f32r, kind="ExternalInput").ap()
    yT_d = nc.dram_tensor("yT", [E, TPC], f32, kind="ExternalOutput").ap()

    qT_d = nc.dram_tensor("qT_scr", [E, TPC], f32).ap()
    kT_d = nc.dram_tensor("kT_scr", [E, TPC], f32).ap()
    vT_d = nc.dram_tensor("vT_scr", [E, TPC], f32).ap()

    NE = E // P   # 16 k-tiles
    NO = E // P   # 16 o-tiles
    SC = 1.0 / math.sqrt(HD)

    with tile.TileContext(nc) as tc, ExitStack() as ctx:
        glob = ctx.enter_context(tc.tile_pool(name="glob", bufs=1))
        ident = glob.tile([P, P], f32)
        make_identity(nc, ident)

        # ============ PASS A: qT/kT/vT = (W @ x.T) -> DRAM ============
        with nc.named_scope("passA"), \
             tc.tile_pool(name="xsb", bufs=1) as xpool, \
             tc.tile_pool(name="wA", bufs=6) as wpool, \
             tc.tile_pool(name="psA", bufs=8, space="PSUM") as pspool, \
             tc.tile_pool(name="stA", bufs=4) as stpool:
            xsb = xpool.tile([P, NE, TPC], f32r)
            for e in range(NE):
                nc.sync.dma_start(out=xsb[:, e, :], in_=xT_d[e * P:(e + 1) * P, :])

            wmats = [WqT_d, WkT_d, WvT_d]
            outs = [qT_d, kT_d, vT_d]
            for oi in range(NO):
                wg = []
                for m in range(3):
                    wt = wpool.tile([P, NE, P], f32r, tag="wA")
                    for e in range(NE):
                        nc.sync.dma_start(
                            out=wt[:, e, :],
                            in_=wmats[m][e * P:(e + 1) * P,
                                         oi * P:(oi + 1) * P])
                    wg.append(wt)
                for tcix in range(TPC // TC):
                    for m in range(3):
                        acc = pspool.tile([P, TC], f32, tag="accA")
                        for e in range(NE):
                            nc.tensor.matmul(
                                acc,
                                wg[m][:, e, :],
                                xsb[:, e, tcix * TC:(tcix + 1) * TC],
                                start=(e == 0), stop=(e == NE - 1))
                        st = stpool.tile([P, TC], f32, tag="stA")
                        nc.vector.tensor_copy(st, acc)
                        nc.sync.dma_start(
                            out=outs[m][oi * P:(oi + 1) * P,
                                        tcix * TC:(tcix + 1) * TC],
                            in_=st)

        import os as _os
        if _os.environ.get("KERNEL_PASS_A_ONLY"):
            # debug: skip pass B entirely (output stays unwritten)
            _skip_b = True
        else:
            _skip_b = False
        # ============ PASS B: attention + Wo ============
        NG = TC // 4           # 4-token groups per chunk
        SUB = 64               # tokens per v2 relayout block
        if _skip_b:
            qkvp = None
        if not _skip_b:
         with nc.named_scope("passB"), \
             tc.tile_pool(name="qkv", bufs=1) as qkvp, \
             tc.tile_pool(name="v2p", bufs=1) as v2p, \
             tc.tile_pool(name="bdp", bufs=1) as bdp, \
             tc.tile_pool(name="vgp", bufs=1) as vgp, \
             tc.tile_pool(name="mid", bufs=4) as mid, \
             tc.tile_pool(name="aop", bufs=2) as aop, \
             tc.tile_pool(name="woP", bufs=2) as woP, \
             tc.tile_pool(name="yst", bufs=3) as yst, \
             tc.tile_pool(name="psS", bufs=2, space="PSUM") as psS, \
             tc.tile_pool(name="psM", bufs=4, space="PSUM") as psM, \
             tc.tile_pool(name="psY", bufs=2, space="PSUM") as psY:

            # persistent manually-rotated slots (stable zero padding)
            NBD = 8
            bd_slots = []
            for i in range(NBD):
                t = bdp.tile([P, 64], f32, tag=f"bd{i}")
                nc.vector.memset(t, 0.0)
                bd_slots.append(t)
            NV2 = 2
            v2_slots = []
            for i in range(NV2):
                t = v2p.tile([P, SUB, 32], f32, tag=f"v2_{i}")
                nc.vector.memset(t, 0.0)
                v2_slots.append(t)
            NVG = 8
            vg_slots = []
            for i in range(NVG):
                t = vgp.tile([P, HD + 1], f32, tag=f"vg{i}")
                nc.vector.memset(t[:, HD:HD + 1], 1.0)
                vg_slots.append(t)

            # Wo matmul stream for chunk c-1, interleaved 2 MMs per middle
            # group of chunk c so the PE never idles long enough to cool.
            wo_seq = [(oi, h) for oi in range(NO) for h in range(NH)]

            def wo_step(state, nsteps):
                for _ in range(nsteps):
                    if state is None or state["pos"] >= len(wo_seq):
                        return
                    oi, h = wo_seq[state["pos"]]
                    state["pos"] += 1
                    if h == 0:
                        wo = woP.tile([P, NH, P], f32r, tag="wo", name="wo")
                        nc.sync.dma_start(
                            out=wo,
                            in_=WoT_d[:, oi * P:(oi + 1) * P]
                            .rearrange("(hh p) o -> p hh o", p=P))
                        state["wo"] = wo
                        state["yp"] = psY.tile([P, TC], f32, tag="yps", name="yps")
                    nc.tensor.matmul(
                        state["yp"], state["wo"][:, h, :],
                        state["aoT"][:, h, :],
                        start=(h == 0), stop=(h == NH - 1))
                    if h == NH - 1:
                        ys = yst.tile([P, TC], f32, tag="ys")
                        nc.vector.tensor_copy(ys, state["yp"])
                        nc.sync.dma_start(
                            out=yT_d[oi * P:(oi + 1) * P,
                                     state["t0"]:state["t0"] + TC],
                            in_=ys)

            gi_all = 0
            v2i = 0
            prev = None
            for tcix in range(TPC // TC):
                t0 = tcix * TC
                q_sb = qkvp.tile([P, NH, TC], f32, tag="q")
                k_sb = qkvp.tile([P, NH, TC], f32, tag="k")
                v_sb = qkvp.tile([P, NH, TC], f32, tag="v")
                for g in range(NH):
                    nc.sync.dma_start(out=q_sb[:, g, :],
                                      in_=qT_d[g * P:(g + 1) * P, t0:t0 + TC])
                    nc.sync.dma_start(out=k_sb[:, g, :],
                                      in_=kT_d[g * P:(g + 1) * P, t0:t0 + TC])
                    nc.sync.dma_start(out=v_sb[:, g, :],
                                      in_=vT_d[g * P:(g + 1) * P, t0:t0 + TC])

                aoT = aop.tile([P, NH, TC], f32r, tag="aoT")

                for sub in range(TC // SUB):
                    # relayout v to token-major with padded 32-col slots
                    v2 = v2_slots[v2i % NV2]
                    v2i += 1
                    nc.gpsimd.tensor_copy(
                        v2[:, :, 0:NH],
                        v_sb[:, :, sub * SUB:(sub + 1) * SUB]
                        .rearrange("p g t -> p t g"))

                    for gi4 in range(SUB // 4):
                        tt = sub * SUB + gi4 * 4   # first token in group
                        bd = bd_slots[gi_all % NBD]
                        vg = vg_slots[gi_all % NVG]
                        gi_all += 1

                        # V block transpose: [128, 4*32] -> [4*32, 128]
                        vg_ps = psM.tile([P, P], f32, tag="mps")
                        nc.tensor.transpose(
                            vg_ps,
                            v2[:, gi4 * 4:(gi4 + 1) * 4, :]
                            .rearrange("p t g -> p (t g)"),
                            ident)
                        nc.vector.tensor_copy(vg[:, 0:HD], vg_ps)

                        # scores for 4 tokens -> one psum tile at 32-strips
                        sc_ps = psS.tile([P, NH], f32, tag="scps")
                        for j in range(4):
                            t = tt + j
                            nc.tensor.matmul(
                                sc_ps[32 * j:32 * j + NH, :],
                                k_sb[:, :, t], q_sb[:, :, t],
                                start=True, stop=True,
                                tile_position=(0, 32 * j))
                        # exp for all 4 tokens in one ACT op, then build the
                        # block-diagonal with gpsimd (idle engine) copies
                        es = mid.tile([P, NH], f32, tag="es")
                        nc.scalar.activation(
                            out=es, in_=sc_ps,
                            func=mybir.ActivationFunctionType.Exp,
                            scale=SC)
                        for j in range(4):
                            nc.gpsimd.tensor_copy(
                                bd[32 * j:32 * j + NH, NH * j:NH * (j + 1)],
                                es[32 * j:32 * j + NH, :])

                        # AV: [64,(t,h)] x [128, d+1]
                        av_ps = psM.tile([P, HD + 1], f32, tag="mps")
                        nc.tensor.matmul(av_ps[0:64, :], bd, vg, start=True, stop=True)

                        invz = mid.tile([64, 1], f32, tag="invz")
                        nc.vector.reciprocal(invz, av_ps[0:64, HD:HD + 1])
                        ao = mid.tile([64, HD], f32, tag="ao")
                        nc.vector.tensor_scalar_mul(ao, av_ps[0:64, 0:HD], invz)

                        # transpose back: [64,(t,h) x 128 d] -> [128 d, 64]
                        aoT_ps = psM.tile([P, 64], f32, tag="mps")
                        nc.tensor.transpose(aoT_ps, ao, ident[0:64, 0:64])
                        nc.vector.tensor_copy(
                            aoT[:, :, tt:tt + 4].rearrange("p h t -> p h t"),
                            aoT_ps.rearrange("p (t h) -> p h t", t=4))
                        wo_step(prev, 2)

                # drain any remainder of the previous chunk's Wo stream
                wo_step(prev, len(wo_seq))
                prev = {"pos": 0, "aoT": aoT, "t0": t0, "wo": None, "yp": None}
            wo_step(prev, len(wo_seq))

    nc.compile()
    return nc


def _get_program():
    if "nc" not in _cached:
        _cached["nc"] = _build_program()
    return _cached["nc"]


def kernel(x, Wq, Wk, Wv, Wo):
    from concourse.bass_utils import run_bass_kernel_spmd

    B, S, H = x.shape
    assert (B * S, H) == (NCORES * TPC, E)
    nc = _get_program()

    xf = np.ascontiguousarray(x.reshape(B * S, H))
    WqT = np.ascontiguousarray(Wq.T)
    WkT = np.ascontiguousarray(Wk.T)
    WvT = np.ascontiguousarray(Wv.T)
    WoT = np.ascontiguousarray(Wo.T)

    in_maps = []
    for i in range(NCORES):
        xT = np.ascontiguousarray(xf[i * TPC:(i + 1) * TPC, :].T)
        in_maps.append({"xT": xT, "WqT": WqT, "WkT": WkT,
                        "WvT": WvT, "WoT": WoT})

    import os
    trace = bool(int(os.environ.get("BASS_KERNEL_TRACE", "0")))
    res = run_bass_kernel_spmd(nc, in_maps, core_ids=list(range(NCORES)),
                               trace=trace)
    if trace:
        _cached["last_results"] = res
    parts = [res.results[i]["yT"].T for i in range(NCORES)]
    y = np.concatenate(parts, axis=0).reshape(B, S, H)
    return np.ascontiguousarray(y.astype(np.float32))



# revision 5
# speedup vs baseline: 1.9933x; 1.9933x over previous
"""Trainium2 Bass kernel for per-token multi-head self-attention.

Computation (per token t):
  q,k,v = x @ W{q,k,v}.T ; scores = (q_t k_t^T)/sqrt(128) over heads [16x16]
  out_t = softmax(scores) @ v_t ; y = out @ Wo.T

Sharding: data-parallel over the 16384 tokens -> 8 cores x 2048 tokens.

Fully-fused single-pass structure, all matmul operands in bf16 (fp32 PSUM
accumulation; CPU-simulated pipeline rel err ~4.4e-3 vs the 2e-2 gate):
  - One stream of "GEMM units" (QKV projection tiles and Wo output tiles)
    is interleaved ("pumped") between the small attention-middle ops so the
    PE never idles on the middle's cross-engine dependency chains.
  - qkv never round-trips through DRAM: QKV units for chunk c+1 run (as
    pump filler) during the attention middle of chunk c, writing SBUF
    double buffers.
  - Attention middle processes 8 tokens per group: per-token 16x16 score
    matmuls (4 PE column-groups x 2 rounds) -> one exp ACT -> block-diag
    [128,128] attn matrix (copies split across gpsimd/vector/scalar) ->
    one AV matmul against the PE-transposed V block with a ones column
    producing the softmax normalizer -> per-partition 1/Z scale -> one
    PE-transpose back to feature-major layout for the Wo GEMM.
Weights are host-side pre-tiled so every weight-tile DMA reads 2-4KB
contiguous runs.
"""
import math
from contextlib import ExitStack

import numpy as np

NCORES = 8
E = 2048          # hidden
NH = 16           # heads
HD = 128          # head dim
TPC = 2048        # tokens per core
TC = 512          # token chunk
P = 128
NE = E // P       # 16 contraction tiles
NO = E // P       # 16 output tiles
NCH = TPC // TC   # 4 chunks
NG = TC // 8      # 64 8-token groups per chunk
SC = 1.0 / math.sqrt(HD)

_cached = {}


def _build_program():
    import concourse.bass as bass
    import concourse.tile as tile
    from concourse import bacc, mybir
    from concourse.masks import make_identity

    f32 = mybir.dt.float32
    bf16 = mybir.dt.bfloat16

    nc = bacc.Bacc("TRN2", target_bir_lowering=False, debug=False)

    xT_d = nc.dram_tensor("xT", [E, TPC], bf16, kind="ExternalInput").ap()
    # pre-tiled weights: row oi*128+p, col e*128+o  (p = input-feature within
    # e-slice for QKV; for Wo: p = head-dim within head h, col h*128+o)
    Wq_d = nc.dram_tensor("Wq", [E, E], bf16, kind="ExternalInput").ap()
    Wk_d = nc.dram_tensor("Wk", [E, E], bf16, kind="ExternalInput").ap()
    Wv_d = nc.dram_tensor("Wv", [E, E], bf16, kind="ExternalInput").ap()
    Wo_d = nc.dram_tensor("Wo", [E, E], bf16, kind="ExternalInput").ap()
    yT_d = nc.dram_tensor("yT", [E, TPC], f32, kind="ExternalOutput").ap()

    with tile.TileContext(nc) as tc, ExitStack() as ctx:
        glob = ctx.enter_context(tc.tile_pool(name="glob", bufs=1))
        ident = glob.tile([P, P], bf16)
        make_identity(nc, ident)

        xp = ctx.enter_context(tc.tile_pool(name="xp", bufs=1))
        qkvp = ctx.enter_context(tc.tile_pool(name="qkvp", bufs=1))
        aotp = ctx.enter_context(tc.tile_pool(name="aotp", bufs=1))
        v2p = ctx.enter_context(tc.tile_pool(name="v2p", bufs=1))
        bdp = ctx.enter_context(tc.tile_pool(name="bdp", bufs=1))
        vgp = ctx.enter_context(tc.tile_pool(name="vgp", bufs=1))
        wp = ctx.enter_context(tc.tile_pool(name="wp", bufs=4))
        esp = ctx.enter_context(tc.tile_pool(name="esp", bufs=3))
        aop = ctx.enter_context(tc.tile_pool(name="aop", bufs=3))
        ivp = ctx.enter_context(tc.tile_pool(name="ivp", bufs=3))
        ysp = ctx.enter_context(tc.tile_pool(name="ysp", bufs=3))
        psG = ctx.enter_context(tc.tile_pool(name="psG", bufs=3, space="PSUM"))
        psS = ctx.enter_context(tc.tile_pool(name="psS", bufs=2, space="PSUM"))
        psM = ctx.enter_context(tc.tile_pool(name="psM", bufs=3, space="PSUM"))

        # persistent double buffers
        xb = [xp.tile([P, NE, TC], bf16, tag=f"x{i}", name=f"x{i}")
              for i in range(2)]
        qkv = [[qkvp.tile([P, NO, TC], bf16, tag=f"qkv{m}_{i}",
                          name=f"qkv{m}_{i}")
                for i in range(2)] for m in range(3)]
        aoT = [aotp.tile([P, NH, TC], bf16, tag=f"aoT{i}", name=f"aoT{i}")
               for i in range(2)]
        v2 = []
        for i in range(2):
            t = v2p.tile([P, 64, 32], bf16, tag=f"v2_{i}", name=f"v2_{i}")
            nc.vector.memset(t, 0.0)
            v2.append(t)
        NBD = 8
        bds = []
        for i in range(NBD):
            t = bdp.tile([P, P], bf16, tag=f"bd{i}", name=f"bd{i}")
            nc.vector.memset(t, 0.0)
            bds.append(t)
        NVG = 8
        vgs = []
        for i in range(NVG):
            t = vgp.tile([P, HD + 1], bf16, tag=f"vg{i}", name=f"vg{i}")
            nc.vector.memset(t, 0.0)
            nc.vector.memset(t[:, HD:HD + 1], 1.0)
            vgs.append(t)

        wmats = [Wq_d, Wk_d, Wv_d]

        def load_x(c):
            for e in range(NE):
                nc.sync.dma_start(
                    out=xb[c % 2][:, e, :],
                    in_=xT_d[e * P:(e + 1) * P, c * TC:(c + 1) * TC])

        # ---------------- GEMM unit machinery ----------------
        # Each unit: (prefetch_fn -> returns w tile, gen_fn(w) yields per MM)
        def qkv_unit(c, oi, m):
            def pre():
                wt = wp.tile([P, NE, P], bf16, tag="w", name="w")
                wf = wt.rearrange("p e o -> p (e o)")
                nc.sync.dma_start(out=wf[:, 0:E // 2],
                                  in_=wmats[m][oi * P:(oi + 1) * P, 0:E // 2])
                nc.sync.dma_start(out=wf[:, E // 2:E],
                                  in_=wmats[m][oi * P:(oi + 1) * P, E // 2:E])
                return wt

            def gen(wt):
                acc = psG.tile([P, TC], f32, tag="acc", name="acc")
                for e in range(NE):
                    nc.tensor.matmul(acc, wt[:, e, :], xb[c % 2][:, e, :],
                                     start=(e == 0), stop=(e == NE - 1))
                    yield
                nc.vector.tensor_copy(qkv[m][c % 2][:, oi, :], acc)

            return pre, gen

        def wo_unit(c, oi):
            def pre():
                wt = wp.tile([P, NH, P], bf16, tag="w", name="w")
                wf = wt.rearrange("p h o -> p (h o)")
                nc.sync.dma_start(out=wf[:, 0:E // 2],
                                  in_=Wo_d[oi * P:(oi + 1) * P, 0:E // 2])
                nc.sync.dma_start(out=wf[:, E // 2:E],
                                  in_=Wo_d[oi * P:(oi + 1) * P, E // 2:E])
                return wt

            def gen(wt):
                yp = psG.tile([P, TC], f32, tag="acc", name="yp")
                for h in range(NH):
                    nc.tensor.matmul(yp, wt[:, h, :], aoT[c % 2][:, h, :],
                                     start=(h == 0), stop=(h == NH - 1))
                    yield
                ys = ysp.tile([P, TC], f32, tag="ys", name="ys")
                nc.vector.tensor_copy(ys, yp)
                nc.sync.dma_start(
                    out=yT_d[oi * P:(oi + 1) * P, c * TC:(c + 1) * TC],
                    in_=ys)

            return pre, gen

        pend = []          # [pre, gen] not yet prefetched
        active = []        # generators with w already fetched
        LOOKAHEAD = 3

        def refill():
            while pend and len(active) < LOOKAHEAD:
                pre, gen = pend.pop(0)
                active.append(gen(pre()))

        def pump(n):
            refill()
            while n > 0 and active:
                g = active[0]
                try:
                    next(g)
                    n -= 1
                except StopIteration:
                    active.pop(0)
                    refill()

        def pump_all():
            refill()
            while active:
                try:
                    next(active[0])
                except StopIteration:
                    active.pop(0)
                    refill()

        # ---------------- attention middle ----------------
        def relayout(c, sub):
            nc.gpsimd.tensor_copy(
                v2[sub % 2][:, :, 0:NH],
                qkv[2][c % 2][:, :, sub * 64:(sub + 1) * 64]
                .rearrange("p g t -> p t g"))

        state = {"gi": 0, "prev": None}

        def phase_a(c, s):
            sub = s // 8
            if s % 8 == 4 and sub + 1 < 8:
                relayout(c, sub + 1)
            q_sb, k_sb = qkv[0][c % 2], qkv[1][c % 2]
            sc = psS.tile([P, 32], f32, tag="sc", name="sc")
            t0 = s * 8
            for j in range(4):
                for half in range(2):
                    t = t0 + 4 * half + j
                    nc.tensor.matmul(
                        sc[32 * j:32 * j + NH, 16 * half:16 * half + 16],
                        k_sb[:, :, t], q_sb[:, :, t],
                        start=True, stop=True,
                        tile_position=(0, 32 * j))
            es = esp.tile([P, 32], bf16, tag="es", name="es")
            nc.scalar.activation(out=es, in_=sc,
                                 func=mybir.ActivationFunctionType.Exp,
                                 scale=SC)
            gi = state["gi"]
            # two 4-token block-diag tiles, rows 32*jloc+g (32-aligned)
            for half in range(2):
                bd = bds[(2 * gi + half) % NBD]
                bdv = bd.rearrange("p (h t) -> p h t", t=8)
                for j in range(4):
                    nc.gpsimd.tensor_copy(
                        bdv[32 * j:32 * j + 16, :, 4 * half + j],
                        es[32 * j:32 * j + 16,
                           16 * half:16 * half + 16])
                # V block transpose: [128 d, 4t*32] -> [(32t+g), d]
                vps = psM.tile([P, P], bf16, tag="m", name="vps")
                nc.tensor.transpose(
                    vps,
                    v2[sub % 2][:, (s % 8) * 8 + 4 * half:
                                (s % 8) * 8 + 4 * half + 4, :]
                    .rearrange("p t g -> p (t g)"),
                    ident)
                vg = vgs[(2 * gi + half) % NVG]
                nc.vector.tensor_copy(vg[:, 0:HD], vps)
            state["gi"] = gi + 1
            state["prev"] = (gi, s)

        def phase_b1(c, prev):
            gi, s = prev
            av = psM.tile([P, HD + 1], f32, tag="m", name="av")
            nc.tensor.matmul(av, bds[(2 * gi) % NBD], vgs[(2 * gi) % NVG],
                             start=True, stop=False)
            nc.tensor.matmul(av, bds[(2 * gi + 1) % NBD],
                             vgs[(2 * gi + 1) % NVG],
                             start=False, stop=True)
            invz = ivp.tile([P, 1], f32, tag="iv", name="invz")
            nc.vector.reciprocal(invz, av[:, HD:HD + 1])
            ao = aop.tile([P, P], bf16, tag="ao", name="ao")
            nc.vector.tensor_scalar_mul(ao, av[:, 0:HD], invz)
            return ao

        def phase_b2(c, prev, ao):
            gi, s = prev
            aops = psM.tile([P, P], bf16, tag="m", name="aops")
            nc.tensor.transpose(aops, ao, ident)
            nc.vector.tensor_copy(
                aoT[c % 2][:, :, 8 * s:8 * s + 8],
                aops.rearrange("p (h t) -> p h t", t=8))

        # ---------------- schedule ----------------
        load_x(0)
        load_x(1)
        for oi in range(NO):
            for m in range(3):
                pend.append(qkv_unit(0, oi, m))
        pump_all()

        for c in range(NCH):
            if c + 2 < NCH:
                load_x(c + 2)
            if c >= 1:
                for oi in range(NO):
                    pend.append(wo_unit(c - 1, oi))
            if c + 1 < NCH:
                for oi in range(NO):
                    for m in range(3):
                        pend.append(qkv_unit(c + 1, oi, m))
            relayout(c, 0)
            state["prev"] = None
            for s in range(NG):
                prev = state["prev"]
                phase_a(c, s)
                pump(6)
                if prev is not None:
                    ao = phase_b1(c, prev)
                    pump(5)
                    phase_b2(c, prev, ao)
                else:
                    pump(5)
                pump(6)
            prev = state["prev"]
            ao = phase_b1(c, prev)
            phase_b2(c, prev, ao)
        for oi in range(NO):
            pend.append(wo_unit(NCH - 1, oi))
        pump_all()

    nc.compile()
    return nc


def _get_program():
    if "nc" not in _cached:
        _cached["nc"] = _build_program()
    return _cached["nc"]


def kernel(x, Wq, Wk, Wv, Wo):
    import ml_dtypes
    from concourse.bass_utils import run_bass_kernel_spmd

    bfd = ml_dtypes.bfloat16
    B, S, H = x.shape
    assert (B * S, H) == (NCORES * TPC, E)
    nc = _get_program()

    def pretile(W):
        # [oi, p, e, o] with row oi*128+p, col e*128+o ; W is [out, in]
        A = np.asarray(W).reshape(NO, P, NE, P).transpose(0, 3, 2, 1)
        return np.ascontiguousarray(A.reshape(E, E).astype(bfd))

    Wqp = pretile(Wq)
    Wkp = pretile(Wk)
    Wvp = pretile(Wv)
    Wop = pretile(Wo)

    xf = np.asarray(x).reshape(B * S, H)
    in_maps = []
    for i in range(NCORES):
        xT = np.ascontiguousarray(
            xf[i * TPC:(i + 1) * TPC, :].T.astype(bfd))
        in_maps.append({"xT": xT, "Wq": Wqp, "Wk": Wkp,
                       "Wv": Wvp, "Wo": Wop})

    import os
    trace = bool(int(os.environ.get("BASS_KERNEL_TRACE", "0")))
    res = run_bass_kernel_spmd(nc, in_maps, core_ids=list(range(NCORES)),
                               trace=trace)
    if trace:
        _cached["last_results"] = res
    parts = [res.results[i]["yT"].T for i in range(NCORES)]
    y = np.concatenate(parts, axis=0).reshape(B, S, H)
    return np.ascontiguousarray(y.astype(np.float32))


# revision 14
# speedup vs baseline: 2.1738x; 1.0906x over previous
"""Trainium2 Bass kernel for per-token multi-head self-attention.

Computation (per token t):
  q,k,v = x @ W{q,k,v}.T ; scores = (q_t k_t^T)/sqrt(128) over heads [16x16]
  out_t = softmax(scores) @ v_t ; y = out @ Wo.T

Sharding: data-parallel over the 16384 tokens -> 8 cores x 2048 tokens.

Fully-fused single-pass structure, all matmul operands in bf16 (fp32 PSUM
accumulation; CPU-simulated pipeline rel err ~4.4e-3 vs the 2e-2 gate):
  - One stream of "GEMM units" (QKV projection tiles and Wo output tiles)
    is interleaved ("pumped") between the small attention-middle ops so the
    PE never idles on the middle's cross-engine dependency chains.
  - qkv never round-trips through DRAM: QKV units for chunk c+1 run (as
    pump filler) during the attention middle of chunk c, writing SBUF
    double buffers.
  - Attention middle processes 8 tokens per group: per-token 16x16 score
    matmuls (4 PE column-groups x 2 rounds) -> one exp ACT -> block-diag
    [128,128] attn matrix (copies split across gpsimd/vector/scalar) ->
    one AV matmul against the PE-transposed V block with a ones column
    producing the softmax normalizer -> per-partition 1/Z scale -> one
    PE-transpose back to feature-major layout for the Wo GEMM.
Weights are host-side pre-tiled so every weight-tile DMA reads 2-4KB
contiguous runs.
"""
import math
from contextlib import ExitStack

import numpy as np

NCORES = 8
E = 2048          # hidden
NH = 16           # heads
HD = 128          # head dim
TPC = 2048        # tokens per core
TC = 512          # token chunk
P = 128
NE = E // P       # 16 contraction tiles
NO = E // P       # 16 output tiles
NCH = TPC // TC   # 4 chunks
NG = TC // 8      # 64 8-token groups per chunk
SC = 1.0 / math.sqrt(HD)

_cached = {}


_dbg = {}


def _build_program():
    import concourse.bass as bass
    import concourse.tile as tile
    from concourse import bacc, mybir
    from concourse.masks import make_identity

    f32 = mybir.dt.float32
    bf16 = mybir.dt.bfloat16

    nc = bacc.Bacc("TRN2", target_bir_lowering=False, debug=False)

    xT_d = nc.dram_tensor("xT", [E, TPC], bf16, kind="ExternalInput").ap()
    # pre-tiled weights: row oi*128+p, col e*128+o  (p = input-feature within
    # e-slice for QKV; for Wo: p = head-dim within head h, col h*128+o)
    Wq_d = nc.dram_tensor("Wq", [E, E], bf16, kind="ExternalInput").ap()
    Wk_d = nc.dram_tensor("Wk", [E, E], bf16, kind="ExternalInput").ap()
    Wv_d = nc.dram_tensor("Wv", [E, E], bf16, kind="ExternalInput").ap()
    Wo_d = nc.dram_tensor("Wo", [E, E], bf16, kind="ExternalInput").ap()
    yT_d = nc.dram_tensor("yT", [E, TPC], f32, kind="ExternalOutput").ap()

    with tile.TileContext(nc) as tc, ExitStack() as ctx:
        glob = ctx.enter_context(tc.tile_pool(name="glob", bufs=1))
        ident = glob.tile([P, P], bf16)
        make_identity(nc, ident)

        xp = ctx.enter_context(tc.tile_pool(name="xp", bufs=1))
        qkvp = ctx.enter_context(tc.tile_pool(name="qkvp", bufs=1))
        aotp = ctx.enter_context(tc.tile_pool(name="aotp", bufs=1))
        v2p = ctx.enter_context(tc.tile_pool(name="v2p", bufs=1))
        bdp = ctx.enter_context(tc.tile_pool(name="bdp", bufs=1))
        vgp = ctx.enter_context(tc.tile_pool(name="vgp", bufs=1))
        wp = ctx.enter_context(tc.tile_pool(name="wp", bufs=4))
        esp = ctx.enter_context(tc.tile_pool(name="esp", bufs=3))
        aop = ctx.enter_context(tc.tile_pool(name="aop", bufs=3))
        ivp = ctx.enter_context(tc.tile_pool(name="ivp", bufs=3))
        ysp = ctx.enter_context(tc.tile_pool(name="ysp", bufs=3))
        psG = ctx.enter_context(tc.tile_pool(name="psG", bufs=3, space="PSUM"))
        psS = ctx.enter_context(tc.tile_pool(name="psS", bufs=2, space="PSUM"))
        psM = ctx.enter_context(tc.tile_pool(name="psM", bufs=3, space="PSUM"))

        # persistent double buffers
        xb = [xp.tile([P, NE, TC], bf16, tag=f"x{i}", name=f"x{i}")
              for i in range(2)]
        qkv = [[qkvp.tile([P, NO, TC], bf16, tag=f"qkv{m}_{i}",
                          name=f"qkv{m}_{i}")
                for i in range(2)] for m in range(3)]
        aoT = [aotp.tile([P, NH, TC], bf16, tag=f"aoT{i}", name=f"aoT{i}")
               for i in range(2)]
        v2 = []
        for i in range(2):
            t = v2p.tile([P, 64, 32], bf16, tag=f"v2_{i}", name=f"v2_{i}")
            nc.vector.memset(t, 0.0)
            v2.append(t)
        NBD = 8
        bds = []
        for i in range(NBD):
            t = bdp.tile([P, P], bf16, tag=f"bd{i}", name=f"bd{i}")
            nc.vector.memset(t, 0.0)
            bds.append(t)
        NVG = 8
        vgs = []
        for i in range(NVG):
            t = vgp.tile([P, HD + 1], bf16, tag=f"vg{i}", name=f"vg{i}")
            nc.vector.memset(t, 0.0)
            nc.vector.memset(t[:, HD:HD + 1], 1.0)
            vgs.append(t)

        wmats = [Wq_d, Wk_d, Wv_d]

        def load_x(c):
            for e in range(NE):
                nc.sync.dma_start(
                    out=xb[c % 2][:, e, :],
                    in_=xT_d[e * P:(e + 1) * P, c * TC:(c + 1) * TC])

        # ---------------- GEMM unit machinery ----------------
        # Each unit: (prefetch_fn -> returns w tile, gen_fn(w) yields per MM)
        def qkv_unit(c, oi, m):
            def pre():
                wt = wp.tile([P, NE, P], bf16, tag="w", name="w")
                wf = wt.rearrange("p e o -> p (e o)")
                nc.sync.dma_start(out=wf[:, 0:E // 2],
                                  in_=wmats[m][oi * P:(oi + 1) * P, 0:E // 2])
                nc.sync.dma_start(out=wf[:, E // 2:E],
                                  in_=wmats[m][oi * P:(oi + 1) * P, E // 2:E])
                return wt

            def gen(wt):
                acc = psG.tile([P, TC], f32, tag="acc", name="acc")
                for e in range(NE):
                    nc.tensor.matmul(acc, wt[:, e, :], xb[c % 2][:, e, :],
                                     start=(e == 0), stop=(e == NE - 1))
                    yield
                nc.scalar.activation(
                    out=qkv[m][c % 2][:, oi, :], in_=acc,
                    func=mybir.ActivationFunctionType.Copy)

            return pre, gen

        def wo_unit(c, oi):
            def pre():
                wt = wp.tile([P, NH, P], bf16, tag="w", name="w")
                wf = wt.rearrange("p h o -> p (h o)")
                nc.sync.dma_start(out=wf[:, 0:E // 2],
                                  in_=Wo_d[oi * P:(oi + 1) * P, 0:E // 2])
                nc.sync.dma_start(out=wf[:, E // 2:E],
                                  in_=Wo_d[oi * P:(oi + 1) * P, E // 2:E])
                return wt

            def gen(wt):
                yp = psG.tile([P, TC], f32, tag="acc", name="yp")
                for h in range(NH):
                    nc.tensor.matmul(yp, wt[:, h, :], aoT[c % 2][:, h, :],
                                     start=(h == 0), stop=(h == NH - 1))
                    yield
                ys = ysp.tile([P, TC], f32, tag="ys", name="ys")
                nc.vector.tensor_copy(ys, yp)
                nc.sync.dma_start(
                    out=yT_d[oi * P:(oi + 1) * P, c * TC:(c + 1) * TC],
                    in_=ys)

            return pre, gen

        pend = []          # [pre, gen] not yet prefetched
        active = []        # generators with w already fetched
        LOOKAHEAD = 3

        def refill():
            while pend and len(active) < LOOKAHEAD:
                pre, gen = pend.pop(0)
                active.append(gen(pre()))

        def pump(n):
            refill()
            while n > 0 and active:
                g = active[0]
                try:
                    next(g)
                    n -= 1
                except StopIteration:
                    active.pop(0)
                    refill()

        def pump_all():
            refill()
            while active:
                try:
                    next(active[0])
                except StopIteration:
                    active.pop(0)
                    refill()

        # ---------------- attention middle ----------------
        def relayout(c, sub):
            nc.gpsimd.tensor_copy(
                v2[sub % 2][:, :, 0:NH],
                qkv[2][c % 2][:, :, sub * 64:(sub + 1) * 64]
                .rearrange("p g t -> p t g"))

        state = {"gi": 0, "prev": None}

        def phase_a(c, s):
            sub = s // 8
            if s % 8 == 4 and sub + 1 < 8:
                relayout(c, sub + 1)
            q_sb, k_sb = qkv[0][c % 2], qkv[1][c % 2]
            sc = psS.tile([P, 32], f32, tag="sc", name="sc")
            t0 = s * 8
            for j in range(4):
                for half in range(2):
                    t = t0 + 4 * half + j
                    nc.tensor.matmul(
                        sc[32 * j:32 * j + NH, 16 * half:16 * half + 16],
                        k_sb[:, :, t], q_sb[:, :, t],
                        start=True, stop=True,
                        tile_position=(0, 32 * j))
            es = esp.tile([P, 32], bf16, tag="es", name="es")
            nc.scalar.activation(out=es, in_=sc,
                                 func=mybir.ActivationFunctionType.Exp,
                                 scale=SC)
            gi = state["gi"]
            # two 4-token block-diag tiles, rows 32*jloc+g (32-aligned)
            for half in range(2):
                bd = bds[(2 * gi + half) % NBD]
                bdv = bd.rearrange("p (h t) -> p h t", t=8)
                for j in range(4):
                    dst = bdv[32 * j:32 * j + 16, :, 4 * half + j]
                    srcv = es[32 * j:32 * j + 16,
                              16 * half:16 * half + 16]
                    k = 4 * half + j
                    if k in (0, 3, 6):
                        nc.gpsimd.tensor_copy(dst, srcv)
                    elif k in (1, 4, 7):
                        nc.scalar.activation(
                            out=dst, in_=srcv,
                            func=mybir.ActivationFunctionType.Copy)
                    else:
                        nc.vector.tensor_copy(dst, srcv)
                # V block transpose: [128 d, 4t*32] -> [(32t+g), d]
                vps = psM.tile([P, P], bf16, tag="m", name="vps")
                nc.tensor.transpose(
                    vps,
                    v2[sub % 2][:, (s % 8) * 8 + 4 * half:
                                (s % 8) * 8 + 4 * half + 4, :]
                    .rearrange("p t g -> p (t g)"),
                    ident)
                vg = vgs[(2 * gi + half) % NVG]
                nc.vector.tensor_copy(vg[:, 0:HD], vps)
            state["gi"] = gi + 1
            state["prev"] = (gi, s)

        def phase_b1(c, prev):
            gi, s = prev
            av = psM.tile([P, HD + 1], f32, tag="m", name="av")
            nc.tensor.matmul(av, bds[(2 * gi) % NBD], vgs[(2 * gi) % NVG],
                             start=True, stop=False)
            nc.tensor.matmul(av, bds[(2 * gi + 1) % NBD],
                             vgs[(2 * gi + 1) % NVG],
                             start=False, stop=True)
            invz = ivp.tile([P, 1], f32, tag="iv", name="invz")
            nc.vector.reciprocal(invz, av[:, HD:HD + 1])
            ao = aop.tile([P, P], bf16, tag="ao", name="ao")
            nc.vector.tensor_scalar_mul(ao, av[:, 0:HD], invz)
            return ao

        def phase_b2(c, prev, ao):
            gi, s = prev
            aops = psM.tile([P, P], bf16, tag="m", name="aops")
            nc.tensor.transpose(aops, ao, ident)
            nc.vector.tensor_copy(
                aoT[c % 2][:, :, 8 * s:8 * s + 8],
                aops.rearrange("p (h t) -> p h t", t=8))

        # ---------------- schedule ----------------
        load_x(0)
        load_x(1)
        for oi in range(NO):
            for m in range(3):
                pend.append(qkv_unit(0, oi, m))
        pump_all()

        for c in range(NCH):
            if c + 2 < NCH:
                load_x(c + 2)
            if c >= 1:
                for oi in range(NO):
                    pend.append(wo_unit(c - 1, oi))
            if c + 1 < NCH:
                for oi in range(NO):
                    for m in range(3):
                        pend.append(qkv_unit(c + 1, oi, m))
            relayout(c, 0)
            state["prev"] = None
            for s in range(NG):
                prev = state["prev"]
                phase_a(c, s)
                pump(6)
                if prev is not None:
                    ao = phase_b1(c, prev)
                    pump(5)
                    phase_b2(c, prev, ao)
                else:
                    pump(5)
                pump(6)
            prev = state["prev"]
            ao = phase_b1(c, prev)
            phase_b2(c, prev, ao)
        for oi in range(NO):
            pend.append(wo_unit(NCH - 1, oi))
        pump_all()

    nc.compile()
    return nc


def _get_program():
    if "nc" not in _cached:
        _cached["nc"] = _build_program()
    return _cached["nc"]


def kernel(x, Wq, Wk, Wv, Wo):
    import ml_dtypes
    from concourse.bass_utils import run_bass_kernel_spmd

    bfd = ml_dtypes.bfloat16
    B, S, H = x.shape
    assert (B * S, H) == (NCORES * TPC, E)
    nc = _get_program()

    def pretile(W):
        # [oi, p, e, o] with row oi*128+p, col e*128+o ; W is [out, in]
        A = np.asarray(W).reshape(NO, P, NE, P).transpose(0, 3, 2, 1)
        return np.ascontiguousarray(A.reshape(E, E).astype(bfd))

    Wqp = pretile(Wq)
    Wkp = pretile(Wk)
    Wvp = pretile(Wv)
    Wop = pretile(Wo)

    xf = np.asarray(x).reshape(B * S, H)
    in_maps = []
    for i in range(NCORES):
        xT = np.ascontiguousarray(
            xf[i * TPC:(i + 1) * TPC, :].T.astype(bfd))
        in_maps.append({"xT": xT, "Wq": Wqp, "Wk": Wkp,
                       "Wv": Wvp, "Wo": Wop})

    import os
    trace = bool(int(os.environ.get("BASS_KERNEL_TRACE", "0")))
    res = run_bass_kernel_spmd(nc, in_maps, core_ids=list(range(NCORES)),
                               trace=trace)
    if trace:
        _cached["last_results"] = res
    parts = [res.results[i]["yT"].T for i in range(NCORES)]
    y = np.concatenate(parts, axis=0).reshape(B, S, H)
    return np.ascontiguousarray(y.astype(np.float32))


# revision 15
# speedup vs baseline: 2.1803x; 1.0030x over previous
"""Trainium2 Bass kernel for per-token multi-head self-attention.

Computation (per token t):
  q,k,v = x @ W{q,k,v}.T ; scores = (q_t k_t^T)/sqrt(128) over heads [16x16]
  out_t = softmax(scores) @ v_t ; y = out @ Wo.T

Sharding: data-parallel over the 16384 tokens -> 8 cores x 2048 tokens.

Fully-fused single-pass structure, all matmul operands in bf16 (fp32 PSUM
accumulation; CPU-simulated pipeline rel err ~4.4e-3 vs the 2e-2 gate):
  - One stream of "GEMM units" (QKV projection tiles and Wo output tiles)
    is interleaved ("pumped") between the small attention-middle ops so the
    PE never idles on the middle's cross-engine dependency chains.
  - qkv never round-trips through DRAM: QKV units for chunk c+1 run (as
    pump filler) during the attention middle of chunk c, writing SBUF
    double buffers.
  - Attention middle processes 8 tokens per group: per-token 16x16 score
    matmuls (4 PE column-groups x 2 rounds) -> one exp ACT -> block-diag
    [128,128] attn matrix (copies split across gpsimd/vector/scalar) ->
    one AV matmul against the PE-transposed V block with a ones column
    producing the softmax normalizer -> per-partition 1/Z scale -> one
    PE-transpose back to feature-major layout for the Wo GEMM.
Weights are host-side pre-tiled so every weight-tile DMA reads 2-4KB
contiguous runs.
"""
import math
from contextlib import ExitStack

import numpy as np

NCORES = 8
E = 2048          # hidden
NH = 16           # heads
HD = 128          # head dim
TPC = 2048        # tokens per core
TC = 512          # token chunk
P = 128
NE = E // P       # 16 contraction tiles
NO = E // P       # 16 output tiles
NCH = TPC // TC   # 4 chunks
NG = TC // 8      # 64 8-token groups per chunk
SC = 1.0 / math.sqrt(HD)

_cached = {}


_dbg = {}


def _build_program():
    import concourse.bass as bass
    import concourse.tile as tile
    from concourse import bacc, mybir
    from concourse.masks import make_identity

    f32 = mybir.dt.float32
    bf16 = mybir.dt.bfloat16

    nc = bacc.Bacc("TRN2", target_bir_lowering=False, debug=False)

    xT_d = nc.dram_tensor("xT", [E, TPC], bf16, kind="ExternalInput").ap()
    # pre-tiled weights: row oi*128+p, col e*128+o  (p = input-feature within
    # e-slice for QKV; for Wo: p = head-dim within head h, col h*128+o)
    Wq_d = nc.dram_tensor("Wq", [E, E], bf16, kind="ExternalInput").ap()
    Wk_d = nc.dram_tensor("Wk", [E, E], bf16, kind="ExternalInput").ap()
    Wv_d = nc.dram_tensor("Wv", [E, E], bf16, kind="ExternalInput").ap()
    Wo_d = nc.dram_tensor("Wo", [E, E], bf16, kind="ExternalInput").ap()
    yT_d = nc.dram_tensor("yT", [E, TPC], f32, kind="ExternalOutput").ap()

    with tile.TileContext(nc) as tc, ExitStack() as ctx:
        glob = ctx.enter_context(tc.tile_pool(name="glob", bufs=1))
        ident = glob.tile([P, P], bf16)
        make_identity(nc, ident)

        xp = ctx.enter_context(tc.tile_pool(name="xp", bufs=1))
        qkvp = ctx.enter_context(tc.tile_pool(name="qkvp", bufs=1))
        aotp = ctx.enter_context(tc.tile_pool(name="aotp", bufs=1))
        v2p = ctx.enter_context(tc.tile_pool(name="v2p", bufs=1))
        bdp = ctx.enter_context(tc.tile_pool(name="bdp", bufs=1))
        vgp = ctx.enter_context(tc.tile_pool(name="vgp", bufs=1))
        wp = ctx.enter_context(tc.tile_pool(name="wp", bufs=4))
        esp = ctx.enter_context(tc.tile_pool(name="esp", bufs=3))
        aop = ctx.enter_context(tc.tile_pool(name="aop", bufs=3))
        ivp = ctx.enter_context(tc.tile_pool(name="ivp", bufs=3))
        ysp = ctx.enter_context(tc.tile_pool(name="ysp", bufs=3))
        psG = ctx.enter_context(tc.tile_pool(name="psG", bufs=3, space="PSUM"))
        psS = ctx.enter_context(tc.tile_pool(name="psS", bufs=2, space="PSUM"))
        psM = ctx.enter_context(tc.tile_pool(name="psM", bufs=3, space="PSUM"))

        # persistent double buffers
        xb = [xp.tile([P, NE, TC], bf16, tag=f"x{i}", name=f"x{i}")
              for i in range(2)]
        qkv = [[qkvp.tile([P, NO, TC], bf16, tag=f"qkv{m}_{i}",
                          name=f"qkv{m}_{i}")
                for i in range(2)] for m in range(3)]
        aoT = [aotp.tile([P, NH, TC], bf16, tag=f"aoT{i}", name=f"aoT{i}")
               for i in range(2)]
        v2 = []
        for i in range(2):
            t = v2p.tile([P, 64, 32], bf16, tag=f"v2_{i}", name=f"v2_{i}")
            nc.vector.memset(t, 0.0)
            v2.append(t)
        NBD = 8
        bds = []
        for i in range(NBD):
            t = bdp.tile([P, P], bf16, tag=f"bd{i}", name=f"bd{i}")
            nc.vector.memset(t, 0.0)
            bds.append(t)
        NVG = 8
        vgs = []
        for i in range(NVG):
            t = vgp.tile([P, HD + 1], bf16, tag=f"vg{i}", name=f"vg{i}")
            nc.vector.memset(t, 0.0)
            nc.vector.memset(t[:, HD:HD + 1], 1.0)
            vgs.append(t)

        wmats = [Wq_d, Wk_d, Wv_d]

        def load_x(c):
            for e in range(NE):
                nc.sync.dma_start(
                    out=xb[c % 2][:, e, :],
                    in_=xT_d[e * P:(e + 1) * P, c * TC:(c + 1) * TC])

        # ---------------- GEMM unit machinery ----------------
        # Each unit: (prefetch_fn -> returns w tile, gen_fn(w) yields per MM)
        def qkv_unit(c, oi, m):
            def pre():
                wt = wp.tile([P, NE, P], bf16, tag="w", name="w")
                wf = wt.rearrange("p e o -> p (e o)")
                nc.sync.dma_start(out=wf[:, 0:E // 2],
                                  in_=wmats[m][oi * P:(oi + 1) * P, 0:E // 2])
                nc.sync.dma_start(out=wf[:, E // 2:E],
                                  in_=wmats[m][oi * P:(oi + 1) * P, E // 2:E])
                return wt

            def gen(wt):
                acc = psG.tile([P, TC], f32, tag="acc", name="acc")
                for e in range(NE):
                    nc.tensor.matmul(acc, wt[:, e, :], xb[c % 2][:, e, :],
                                     start=(e == 0), stop=(e == NE - 1))
                    yield
                nc.scalar.activation(
                    out=qkv[m][c % 2][:, oi, :], in_=acc,
                    func=mybir.ActivationFunctionType.Copy)

            return pre, gen

        def wo_unit(c, oi):
            def pre():
                wt = wp.tile([P, NH, P], bf16, tag="w", name="w")
                wf = wt.rearrange("p h o -> p (h o)")
                nc.sync.dma_start(out=wf[:, 0:E // 2],
                                  in_=Wo_d[oi * P:(oi + 1) * P, 0:E // 2])
                nc.sync.dma_start(out=wf[:, E // 2:E],
                                  in_=Wo_d[oi * P:(oi + 1) * P, E // 2:E])
                return wt

            def gen(wt):
                yp = psG.tile([P, TC], f32, tag="acc", name="yp")
                for h in range(NH):
                    nc.tensor.matmul(yp, wt[:, h, :], aoT[c % 2][:, h, :],
                                     start=(h == 0), stop=(h == NH - 1))
                    yield
                ys = ysp.tile([P, TC], f32, tag="ys", name="ys")
                nc.vector.tensor_copy(ys, yp)
                nc.sync.dma_start(
                    out=yT_d[oi * P:(oi + 1) * P, c * TC:(c + 1) * TC],
                    in_=ys)

            return pre, gen

        pend = []          # [pre, gen] not yet prefetched
        active = []        # generators with w already fetched
        LOOKAHEAD = 4

        def refill():
            while pend and len(active) < LOOKAHEAD:
                pre, gen = pend.pop(0)
                active.append(gen(pre()))

        def pump(n):
            refill()
            while n > 0 and active:
                g = active[0]
                try:
                    next(g)
                    n -= 1
                except StopIteration:
                    active.pop(0)
                    refill()

        def pump_all():
            refill()
            while active:
                try:
                    next(active[0])
                except StopIteration:
                    active.pop(0)
                    refill()

        # ---------------- attention middle ----------------
        def relayout(c, sub):
            nc.gpsimd.tensor_copy(
                v2[sub % 2][:, :, 0:NH],
                qkv[2][c % 2][:, :, sub * 64:(sub + 1) * 64]
                .rearrange("p g t -> p t g"))

        state = {"gi": 0, "prev": None}

        def phase_a(c, s):
            sub = s // 8
            if s % 8 == 2 and sub + 1 < 8:
                relayout(c, sub + 1)
            q_sb, k_sb = qkv[0][c % 2], qkv[1][c % 2]
            sc = psS.tile([P, 32], f32, tag="sc", name="sc")
            t0 = s * 8
            for j in range(4):
                for half in range(2):
                    t = t0 + 4 * half + j
                    nc.tensor.matmul(
                        sc[32 * j:32 * j + NH, 16 * half:16 * half + 16],
                        k_sb[:, :, t], q_sb[:, :, t],
                        start=True, stop=True,
                        tile_position=(0, 32 * j))
            es = esp.tile([P, 32], bf16, tag="es", name="es")
            nc.scalar.activation(out=es, in_=sc,
                                 func=mybir.ActivationFunctionType.Exp,
                                 scale=SC)
            gi = state["gi"]
            # two 4-token block-diag tiles, rows 32*jloc+g (32-aligned)
            for half in range(2):
                bd = bds[(2 * gi + half) % NBD]
                bdv = bd.rearrange("p (h t) -> p h t", t=8)
                for j in range(4):
                    dst = bdv[32 * j:32 * j + 16, :, 4 * half + j]
                    srcv = es[32 * j:32 * j + 16,
                              16 * half:16 * half + 16]
                    k = 4 * half + j
                    if k in (0, 3, 6):
                        nc.gpsimd.tensor_copy(dst, srcv)
                    elif k in (1, 4, 7):
                        nc.scalar.activation(
                            out=dst, in_=srcv,
                            func=mybir.ActivationFunctionType.Copy)
                    else:
                        nc.vector.tensor_copy(dst, srcv)
                # V block transpose: [128 d, 4t*32] -> [(32t+g), d]
                vps = psM.tile([P, P], bf16, tag="m", name="vps")
                nc.tensor.transpose(
                    vps,
                    v2[sub % 2][:, (s % 8) * 8 + 4 * half:
                                (s % 8) * 8 + 4 * half + 4, :]
                    .rearrange("p t g -> p (t g)"),
                    ident)
                vg = vgs[(2 * gi + half) % NVG]
                nc.vector.tensor_copy(vg[:, 0:HD], vps)
            state["gi"] = gi + 1
            state["prev"] = (gi, s)

        def phase_b1(c, prev):
            gi, s = prev
            av = psM.tile([P, HD + 1], f32, tag="m", name="av")
            nc.tensor.matmul(av, bds[(2 * gi) % NBD], vgs[(2 * gi) % NVG],
                             start=True, stop=False)
            nc.tensor.matmul(av, bds[(2 * gi + 1) % NBD],
                             vgs[(2 * gi + 1) % NVG],
                             start=False, stop=True)
            invz = ivp.tile([P, 1], f32, tag="iv", name="invz")
            nc.vector.reciprocal(invz, av[:, HD:HD + 1])
            ao = aop.tile([P, P], bf16, tag="ao", name="ao")
            nc.vector.tensor_scalar_mul(ao, av[:, 0:HD], invz)
            return ao

        def phase_b2(c, prev, ao):
            gi, s = prev
            aops = psM.tile([P, P], bf16, tag="m", name="aops")
            nc.tensor.transpose(aops, ao, ident)
            nc.vector.tensor_copy(
                aoT[c % 2][:, :, 8 * s:8 * s + 8],
                aops.rearrange("p (h t) -> p h t", t=8))

        # ---------------- schedule ----------------
        load_x(0)
        load_x(1)
        for oi in range(NO):
            for m in range(3):
                pend.append(qkv_unit(0, oi, m))
        pump_all()

        for c in range(NCH):
            if c + 2 < NCH:
                load_x(c + 2)
            if c >= 1:
                for oi in range(NO):
                    pend.append(wo_unit(c - 1, oi))
            if c + 1 < NCH:
                for oi in range(NO):
                    for m in range(3):
                        pend.append(qkv_unit(c + 1, oi, m))
            relayout(c, 0)
            state["prev"] = None
            for s in range(NG):
                prev = state["prev"]
                phase_a(c, s)
                pump(6)
                if prev is not None:
                    ao = phase_b1(c, prev)
                    pump(5)
                    phase_b2(c, prev, ao)
                else:
                    pump(5)
                pump(6)
            prev = state["prev"]
            ao = phase_b1(c, prev)
            phase_b2(c, prev, ao)
        for oi in range(NO):
            pend.append(wo_unit(NCH - 1, oi))
        pump_all()

    nc.compile()
    return nc


def _get_program():
    if "nc" not in _cached:
        _cached["nc"] = _build_program()
    return _cached["nc"]


def kernel(x, Wq, Wk, Wv, Wo):
    import ml_dtypes
    from concourse.bass_utils import run_bass_kernel_spmd

    bfd = ml_dtypes.bfloat16
    B, S, H = x.shape
    assert (B * S, H) == (NCORES * TPC, E)
    nc = _get_program()

    def pretile(W):
        # [oi, p, e, o] with row oi*128+p, col e*128+o ; W is [out, in]
        A = np.asarray(W).reshape(NO, P, NE, P).transpose(0, 3, 2, 1)
        return np.ascontiguousarray(A.reshape(E, E).astype(bfd))

    Wqp = pretile(Wq)
    Wkp = pretile(Wk)
    Wvp = pretile(Wv)
    Wop = pretile(Wo)

    xf = np.asarray(x).reshape(B * S, H)
    in_maps = []
    for i in range(NCORES):
        xT = np.ascontiguousarray(
            xf[i * TPC:(i + 1) * TPC, :].T.astype(bfd))
        in_maps.append({"xT": xT, "Wq": Wqp, "Wk": Wkp,
                       "Wv": Wvp, "Wo": Wop})

    import os
    trace = bool(int(os.environ.get("BASS_KERNEL_TRACE", "0")))
    res = run_bass_kernel_spmd(nc, in_maps, core_ids=list(range(NCORES)),
                               trace=trace)
    if trace:
        _cached["last_results"] = res
    parts = [res.results[i]["yT"].T for i in range(NCORES)]
    y = np.concatenate(parts, axis=0).reshape(B, S, H)
    return np.ascontiguousarray(y.astype(np.float32))


# revision 16
# speedup vs baseline: 2.2610x; 1.0370x over previous
"""Trainium2 Bass kernel for per-token multi-head self-attention.

Computation (per token t):
  q,k,v = x @ W{q,k,v}.T ; scores = (q_t k_t^T)/sqrt(128) over heads [16x16]
  out_t = softmax(scores) @ v_t ; y = out @ Wo.T

Sharding: data-parallel over the 16384 tokens -> 8 cores x 2048 tokens.

Fully-fused single-pass structure, all matmul operands in bf16 (fp32 PSUM
accumulation; CPU-simulated pipeline rel err ~4.4e-3 vs the 2e-2 gate):
  - One stream of "GEMM units" (QKV projection tiles and Wo output tiles)
    is interleaved ("pumped") between the small attention-middle ops so the
    PE never idles on the middle's cross-engine dependency chains.
  - qkv never round-trips through DRAM: QKV units for chunk c+1 run (as
    pump filler) during the attention middle of chunk c, writing SBUF
    double buffers.
  - Attention middle processes 8 tokens per group: per-token 16x16 score
    matmuls (4 PE column-groups x 2 rounds) -> one exp ACT -> block-diag
    [128,128] attn matrix (copies split across gpsimd/vector/scalar) ->
    one AV matmul against the PE-transposed V block with a ones column
    producing the softmax normalizer -> per-partition 1/Z scale -> one
    PE-transpose back to feature-major layout for the Wo GEMM.
Weights are host-side pre-tiled so every weight-tile DMA reads 2-4KB
contiguous runs.
"""
import math
from contextlib import ExitStack

import numpy as np

NCORES = 8
E = 2048          # hidden
NH = 16           # heads
HD = 128          # head dim
TPC = 2048        # tokens per core
TC = 512          # token chunk
P = 128
NE = E // P       # 16 contraction tiles
NO = E // P       # 16 output tiles
NCH = TPC // TC   # 4 chunks
NG = TC // 8      # 64 8-token groups per chunk
SC = 1.0 / math.sqrt(HD)

_cached = {}


_dbg = {}


def _build_program():
    import concourse.bass as bass
    import concourse.tile as tile
    from concourse import bacc, mybir
    from concourse.masks import make_identity

    f32 = mybir.dt.float32
    bf16 = mybir.dt.bfloat16

    nc = bacc.Bacc("TRN2", target_bir_lowering=False, debug=False)

    xT_d = nc.dram_tensor("xT", [E, TPC], bf16, kind="ExternalInput").ap()
    # pre-tiled weights: row oi*128+p, col e*128+o  (p = input-feature within
    # e-slice for QKV; for Wo: p = head-dim within head h, col h*128+o)
    Wq_d = nc.dram_tensor("Wq", [E, E], bf16, kind="ExternalInput").ap()
    Wk_d = nc.dram_tensor("Wk", [E, E], bf16, kind="ExternalInput").ap()
    Wv_d = nc.dram_tensor("Wv", [E, E], bf16, kind="ExternalInput").ap()
    Wo_d = nc.dram_tensor("Wo", [E, E], bf16, kind="ExternalInput").ap()
    yT_d = nc.dram_tensor("yT", [E, TPC], f32, kind="ExternalOutput").ap()

    with tile.TileContext(nc) as tc, ExitStack() as ctx:
        glob = ctx.enter_context(tc.tile_pool(name="glob", bufs=1))
        ident = glob.tile([P, P], bf16)
        make_identity(nc, ident)

        xp = ctx.enter_context(tc.tile_pool(name="xp", bufs=1))
        qkvp = ctx.enter_context(tc.tile_pool(name="qkvp", bufs=1))
        aotp = ctx.enter_context(tc.tile_pool(name="aotp", bufs=1))
        v2p = ctx.enter_context(tc.tile_pool(name="v2p", bufs=1))
        bdp = ctx.enter_context(tc.tile_pool(name="bdp", bufs=1))
        vgp = ctx.enter_context(tc.tile_pool(name="vgp", bufs=1))
        wp = ctx.enter_context(tc.tile_pool(name="wp", bufs=4))
        esp = ctx.enter_context(tc.tile_pool(name="esp", bufs=3))
        aop = ctx.enter_context(tc.tile_pool(name="aop", bufs=3))
        ivp = ctx.enter_context(tc.tile_pool(name="ivp", bufs=3))
        ysp = ctx.enter_context(tc.tile_pool(name="ysp", bufs=3))
        psG = ctx.enter_context(tc.tile_pool(name="psG", bufs=3, space="PSUM"))
        psS = ctx.enter_context(tc.tile_pool(name="psS", bufs=2, space="PSUM"))
        psM = ctx.enter_context(tc.tile_pool(name="psM", bufs=3, space="PSUM"))

        # persistent double buffers
        xb = [xp.tile([P, NE, TC], bf16, tag=f"x{i}", name=f"x{i}")
              for i in range(2)]
        qkv = [[qkvp.tile([P, NO, TC], bf16, tag=f"qkv{m}_{i}",
                          name=f"qkv{m}_{i}")
                for i in range(2)] for m in range(3)]
        aoT = [aotp.tile([P, NH, TC], bf16, tag=f"aoT{i}", name=f"aoT{i}")
               for i in range(2)]
        v2 = []
        for i in range(2):
            t = v2p.tile([P, 64, 32], bf16, tag=f"v2_{i}", name=f"v2_{i}")
            nc.vector.memset(t, 0.0)
            v2.append(t)
        NBD = 4
        bds = []
        for i in range(NBD):
            t = bdp.tile([P, 280], bf16, tag=f"bd{i}", name=f"bd{i}")
            nc.vector.memset(t, 0.0)
            bds.append(t)
        NVG = 8
        vgs = []
        for i in range(NVG):
            t = vgp.tile([P, HD + 1], bf16, tag=f"vg{i}", name=f"vg{i}")
            nc.vector.memset(t, 0.0)
            nc.vector.memset(t[:, HD:HD + 1], 1.0)
            vgs.append(t)

        wmats = [Wq_d, Wk_d, Wv_d]

        def load_x(c):
            for e in range(NE):
                nc.sync.dma_start(
                    out=xb[c % 2][:, e, :],
                    in_=xT_d[e * P:(e + 1) * P, c * TC:(c + 1) * TC])

        # ---------------- GEMM unit machinery ----------------
        # Each unit: (prefetch_fn -> returns w tile, gen_fn(w) yields per MM)
        def qkv_unit(c, oi, m):
            def pre():
                wt = wp.tile([P, NE, P], bf16, tag="w", name="w")
                wf = wt.rearrange("p e o -> p (e o)")
                nc.sync.dma_start(out=wf[:, 0:E // 2],
                                  in_=wmats[m][oi * P:(oi + 1) * P, 0:E // 2])
                nc.sync.dma_start(out=wf[:, E // 2:E],
                                  in_=wmats[m][oi * P:(oi + 1) * P, E // 2:E])
                return wt

            def gen(wt):
                acc = psG.tile([P, TC], f32, tag="acc", name="acc")
                for e in range(NE):
                    nc.tensor.matmul(acc, wt[:, e, :], xb[c % 2][:, e, :],
                                     start=(e == 0), stop=(e == NE - 1))
                    yield
                nc.scalar.activation(
                    out=qkv[m][c % 2][:, oi, :], in_=acc,
                    func=mybir.ActivationFunctionType.Copy)

            return pre, gen

        def wo_unit(c, oi):
            def pre():
                wt = wp.tile([P, NH, P], bf16, tag="w", name="w")
                wf = wt.rearrange("p h o -> p (h o)")
                nc.sync.dma_start(out=wf[:, 0:E // 2],
                                  in_=Wo_d[oi * P:(oi + 1) * P, 0:E // 2])
                nc.sync.dma_start(out=wf[:, E // 2:E],
                                  in_=Wo_d[oi * P:(oi + 1) * P, E // 2:E])
                return wt

            def gen(wt):
                yp = psG.tile([P, TC], f32, tag="acc", name="yp")
                for h in range(NH):
                    nc.tensor.matmul(yp, wt[:, h, :], aoT[c % 2][:, h, :],
                                     start=(h == 0), stop=(h == NH - 1))
                    yield
                ys = ysp.tile([P, TC], f32, tag="ys", name="ys")
                nc.vector.tensor_copy(ys, yp)
                nc.sync.dma_start(
                    out=yT_d[oi * P:(oi + 1) * P, c * TC:(c + 1) * TC],
                    in_=ys)

            return pre, gen

        pend = []          # [pre, gen] not yet prefetched
        active = []        # generators with w already fetched
        LOOKAHEAD = 4

        def refill():
            while pend and len(active) < LOOKAHEAD:
                pre, gen = pend.pop(0)
                active.append(gen(pre()))

        def pump(n):
            refill()
            while n > 0 and active:
                g = active[0]
                try:
                    next(g)
                    n -= 1
                except StopIteration:
                    active.pop(0)
                    refill()

        def pump_all():
            refill()
            while active:
                try:
                    next(active[0])
                except StopIteration:
                    active.pop(0)
                    refill()

        # ---------------- attention middle ----------------
        def relayout(c, sub, half):
            t0 = sub * 64 + 32 * half
            nc.gpsimd.tensor_copy(
                v2[sub % 2][:, 32 * half:32 * (half + 1), 0:NH],
                qkv[2][c % 2][:, :, t0:t0 + 32]
                .rearrange("p g t -> p t g"))

        state = {"gi": 0, "prev": None}

        def phase_a(c, s):
            sub = s // 8
            if s % 8 == 2 and sub + 1 < 8:
                relayout(c, sub + 1, 0)
            if s % 8 == 5 and sub + 1 < 8:
                relayout(c, sub + 1, 1)
            q_sb, k_sb = qkv[0][c % 2], qkv[1][c % 2]
            sc = psS.tile([P, 32], f32, tag="sc", name="sc")
            t0 = s * 8
            for j in range(4):
                for half in range(2):
                    t = t0 + 4 * half + j
                    nc.tensor.matmul(
                        sc[32 * j:32 * j + NH, 16 * half:16 * half + 16],
                        k_sb[:, :, t], q_sb[:, :, t],
                        start=True, stop=True,
                        tile_position=(0, 32 * j))
            es = esp.tile([P, 32], bf16, tag="es", name="es")
            nc.scalar.activation(out=es, in_=sc,
                                 func=mybir.ActivationFunctionType.Exp,
                                 scale=SC)
            gi = state["gi"]
            # one wide block-diag tile: col 140*hf + 8h + j, rows 32j+g
            bd = bds[gi % NBD]
            bdv = bd.rearrange("p (hf q) -> p hf q", hf=2)
            esv = es.rearrange("p (hf g) -> p hf g", hf=2)
            for j in range(4):
                dst = bdv[32 * j:32 * j + 16, :, j:j + 121:8]
                srcv = esv[32 * j:32 * j + 16, :, :]
                if j in (0, 3):
                    nc.gpsimd.tensor_copy(dst, srcv)
                elif j == 1:
                    nc.scalar.activation(
                        out=dst, in_=srcv,
                        func=mybir.ActivationFunctionType.Copy)
                else:
                    nc.vector.tensor_copy(dst, srcv)
            for half in range(2):
                # V block transpose: [128 d, 4t*32] -> [(32t+g), d]
                vps = psM.tile([P, P], bf16, tag="m", name="vps")
                nc.tensor.transpose(
                    vps,
                    v2[sub % 2][:, (s % 8) * 8 + 4 * half:
                                (s % 8) * 8 + 4 * half + 4, :]
                    .rearrange("p t g -> p (t g)"),
                    ident)
                vg = vgs[(2 * gi + half) % NVG]
                nc.vector.tensor_copy(vg[:, 0:HD], vps)
            state["gi"] = gi + 1
            state["prev"] = (gi, s)

        def phase_b1(c, prev):
            gi, s = prev
            av = psM.tile([P, HD + 1], f32, tag="m", name="av")
            bd = bds[gi % NBD]
            nc.tensor.matmul(av, bd[:, 0:P], vgs[(2 * gi) % NVG],
                             start=True, stop=False)
            nc.tensor.matmul(av, bd[:, 136:136 + P],
                             vgs[(2 * gi + 1) % NVG],
                             start=False, stop=True)
            invz = ivp.tile([P, 1], f32, tag="iv", name="invz")
            nc.vector.reciprocal(invz, av[:, HD:HD + 1])
            ao = aop.tile([P, P], bf16, tag="ao", name="ao")
            nc.vector.tensor_scalar_mul(ao, av[:, 0:HD], invz)
            return ao

        def phase_b2(c, prev, ao):
            gi, s = prev
            aops = psM.tile([P, P], bf16, tag="m", name="aops")
            nc.tensor.transpose(aops, ao, ident)
            nc.vector.tensor_copy(
                aoT[c % 2][:, :, 8 * s:8 * s + 8],
                aops.rearrange("p (h t) -> p h t", t=8))

        # ---------------- schedule ----------------
        load_x(0)
        load_x(1)
        for oi in range(NO):
            for m in range(3):
                pend.append(qkv_unit(0, oi, m))
        pump_all()

        for c in range(NCH):
            if c + 2 < NCH:
                load_x(c + 2)
            if c >= 1:
                for oi in range(NO):
                    pend.append(wo_unit(c - 1, oi))
            if c + 1 < NCH:
                for oi in range(NO):
                    for m in range(3):
                        pend.append(qkv_unit(c + 1, oi, m))
            relayout(c, 0, 0)
            relayout(c, 0, 1)
            state["prev"] = None
            for s in range(NG):
                prev = state["prev"]
                phase_a(c, s)
                pump(6)
                if prev is not None:
                    ao = phase_b1(c, prev)
                    pump(5)
                    phase_b2(c, prev, ao)
                else:
                    pump(5)
                pump(6)
            prev = state["prev"]
            ao = phase_b1(c, prev)
            phase_b2(c, prev, ao)
        for oi in range(NO):
            pend.append(wo_unit(NCH - 1, oi))
        pump_all()

    nc.compile()
    return nc


def _get_program():
    if "nc" not in _cached:
        _cached["nc"] = _build_program()
    return _cached["nc"]


def kernel(x, Wq, Wk, Wv, Wo):
    import ml_dtypes
    from concourse.bass_utils import run_bass_kernel_spmd

    bfd = ml_dtypes.bfloat16
    B, S, H = x.shape
    assert (B * S, H) == (NCORES * TPC, E)
    nc = _get_program()

    def pretile(W):
        # [oi, p, e, o] with row oi*128+p, col e*128+o ; W is [out, in]
        A = np.asarray(W).reshape(NO, P, NE, P).transpose(0, 3, 2, 1)
        return np.ascontiguousarray(A.reshape(E, E).astype(bfd))

    Wqp = pretile(Wq)
    Wkp = pretile(Wk)
    Wvp = pretile(Wv)
    Wop = pretile(Wo)

    xf = np.asarray(x).reshape(B * S, H)
    in_maps = []
    for i in range(NCORES):
        xT = np.ascontiguousarray(
            xf[i * TPC:(i + 1) * TPC, :].T.astype(bfd))
        in_maps.append({"xT": xT, "Wq": Wqp, "Wk": Wkp,
                       "Wv": Wvp, "Wo": Wop})

    import os
    trace = bool(int(os.environ.get("BASS_KERNEL_TRACE", "0")))
    res = run_bass_kernel_spmd(nc, in_maps, core_ids=list(range(NCORES)),
                               trace=trace)
    if trace:
        _cached["last_results"] = res
    parts = [res.results[i]["yT"].T for i in range(NCORES)]
    y = np.concatenate(parts, axis=0).reshape(B, S, H)
    return np.ascontiguousarray(y.astype(np.float32))


# revision 17
# speedup vs baseline: 2.3091x; 1.0212x over previous
"""Trainium2 Bass kernel for per-token multi-head self-attention.

Computation (per token t):
  q,k,v = x @ W{q,k,v}.T ; scores = (q_t k_t^T)/sqrt(128) over heads [16x16]
  out_t = softmax(scores) @ v_t ; y = out @ Wo.T

Sharding: data-parallel over the 16384 tokens -> 8 cores x 2048 tokens.

Fully-fused single-pass structure, all matmul operands in bf16 (fp32 PSUM
accumulation; CPU-simulated pipeline rel err ~4.4e-3 vs the 2e-2 gate):
  - One stream of "GEMM units" (QKV projection tiles and Wo output tiles)
    is interleaved ("pumped") between the small attention-middle ops so the
    PE never idles on the middle's cross-engine dependency chains.
  - qkv never round-trips through DRAM: QKV units for chunk c+1 run (as
    pump filler) during the attention middle of chunk c, writing SBUF
    double buffers.
  - Attention middle processes 8 tokens per group: per-token 16x16 score
    matmuls (4 PE column-groups x 2 rounds) -> one exp ACT -> block-diag
    [128,128] attn matrix (copies split across gpsimd/vector/scalar) ->
    one AV matmul against the PE-transposed V block with a ones column
    producing the softmax normalizer -> per-partition 1/Z scale -> one
    PE-transpose back to feature-major layout for the Wo GEMM.
Weights are host-side pre-tiled so every weight-tile DMA reads 2-4KB
contiguous runs.
"""
import math
from contextlib import ExitStack

import numpy as np

NCORES = 8
E = 2048          # hidden
NH = 16           # heads
HD = 128          # head dim
TPC = 2048        # tokens per core
TC = 512          # token chunk
P = 128
NE = E // P       # 16 contraction tiles
NO = E // P       # 16 output tiles
NCH = TPC // TC   # 4 chunks
NG = TC // 8      # 64 8-token groups per chunk
SC = 1.0 / math.sqrt(HD)

_cached = {}


_dbg = {}


def _build_program():
    import concourse.bass as bass
    import concourse.tile as tile
    from concourse import bacc, mybir
    from concourse.masks import make_identity

    f32 = mybir.dt.float32
    bf16 = mybir.dt.bfloat16

    nc = bacc.Bacc("TRN2", target_bir_lowering=False, debug=False)

    xT_d = nc.dram_tensor("xT", [E, TPC], bf16, kind="ExternalInput").ap()
    # pre-tiled weights: row oi*128+p, col e*128+o  (p = input-feature within
    # e-slice for QKV; for Wo: p = head-dim within head h, col h*128+o)
    Wq_d = nc.dram_tensor("Wq", [E, E], bf16, kind="ExternalInput").ap()
    Wk_d = nc.dram_tensor("Wk", [E, E], bf16, kind="ExternalInput").ap()
    Wv_d = nc.dram_tensor("Wv", [E, E], bf16, kind="ExternalInput").ap()
    Wo_d = nc.dram_tensor("Wo", [E, E], bf16, kind="ExternalInput").ap()
    yT_d = nc.dram_tensor("yT", [E, TPC], f32, kind="ExternalOutput").ap()

    with tile.TileContext(nc) as tc, ExitStack() as ctx:
        glob = ctx.enter_context(tc.tile_pool(name="glob", bufs=1))
        ident = glob.tile([P, P], bf16)
        make_identity(nc, ident)

        xp = ctx.enter_context(tc.tile_pool(name="xp", bufs=1))
        qkvp = ctx.enter_context(tc.tile_pool(name="qkvp", bufs=1))
        aotp = ctx.enter_context(tc.tile_pool(name="aotp", bufs=1))
        v2p = ctx.enter_context(tc.tile_pool(name="v2p", bufs=1))
        bdp = ctx.enter_context(tc.tile_pool(name="bdp", bufs=1))
        vgp = ctx.enter_context(tc.tile_pool(name="vgp", bufs=1))
        wp = ctx.enter_context(tc.tile_pool(name="wp", bufs=4))
        esp = ctx.enter_context(tc.tile_pool(name="esp", bufs=3))
        aop = ctx.enter_context(tc.tile_pool(name="aop", bufs=3))
        ivp = ctx.enter_context(tc.tile_pool(name="ivp", bufs=3))
        ysp = ctx.enter_context(tc.tile_pool(name="ysp", bufs=3))
        psG = ctx.enter_context(tc.tile_pool(name="psG", bufs=3, space="PSUM"))
        psS = ctx.enter_context(tc.tile_pool(name="psS", bufs=1, space="PSUM"))
        psM = ctx.enter_context(tc.tile_pool(name="psM", bufs=4, space="PSUM"))

        # persistent double buffers
        xb = [xp.tile([P, NE, TC], bf16, tag=f"x{i}", name=f"x{i}")
              for i in range(2)]
        qkv = [[qkvp.tile([P, NO, TC], bf16, tag=f"qkv{m}_{i}",
                          name=f"qkv{m}_{i}")
                for i in range(2)] for m in range(3)]
        aoT = [aotp.tile([P, NH, TC], bf16, tag=f"aoT{i}", name=f"aoT{i}")
               for i in range(2)]
        v2 = []
        for i in range(2):
            t = v2p.tile([P, 64, 32], bf16, tag=f"v2_{i}", name=f"v2_{i}")
            nc.vector.memset(t, 0.0)
            v2.append(t)
        NBD = 4
        bds = []
        for i in range(NBD):
            t = bdp.tile([P, 280], bf16, tag=f"bd{i}", name=f"bd{i}")
            nc.vector.memset(t, 0.0)
            bds.append(t)
        NVG = 8
        vgs = []
        for i in range(NVG):
            t = vgp.tile([P, HD + 1], bf16, tag=f"vg{i}", name=f"vg{i}")
            nc.vector.memset(t, 0.0)
            nc.vector.memset(t[:, HD:HD + 1], 1.0)
            vgs.append(t)

        wmats = [Wq_d, Wk_d, Wv_d]

        def load_x(c):
            for e in range(NE):
                nc.sync.dma_start(
                    out=xb[c % 2][:, e, :],
                    in_=xT_d[e * P:(e + 1) * P, c * TC:(c + 1) * TC])

        # ---------------- GEMM unit machinery ----------------
        # Each unit: (prefetch_fn -> returns w tile, gen_fn(w) yields per MM)
        def qkv_unit(c, oi, m):
            def pre():
                wt = wp.tile([P, NE, P], bf16, tag="w", name="w")
                wf = wt.rearrange("p e o -> p (e o)")
                nc.sync.dma_start(out=wf[:, 0:E // 2],
                                  in_=wmats[m][oi * P:(oi + 1) * P, 0:E // 2])
                nc.sync.dma_start(out=wf[:, E // 2:E],
                                  in_=wmats[m][oi * P:(oi + 1) * P, E // 2:E])
                return wt

            def gen(wt):
                acc = psG.tile([P, TC], f32, tag="acc", name="acc")
                for e in range(NE):
                    nc.tensor.matmul(acc, wt[:, e, :], xb[c % 2][:, e, :],
                                     start=(e == 0), stop=(e == NE - 1))
                    yield
                nc.scalar.activation(
                    out=qkv[m][c % 2][:, oi, :], in_=acc,
                    func=mybir.ActivationFunctionType.Copy)

            return pre, gen

        def wo_unit(c, oi):
            def pre():
                wt = wp.tile([P, NH, P], bf16, tag="w", name="w")
                wf = wt.rearrange("p h o -> p (h o)")
                nc.sync.dma_start(out=wf[:, 0:E // 2],
                                  in_=Wo_d[oi * P:(oi + 1) * P, 0:E // 2])
                nc.sync.dma_start(out=wf[:, E // 2:E],
                                  in_=Wo_d[oi * P:(oi + 1) * P, E // 2:E])
                return wt

            def gen(wt):
                yp = psG.tile([P, TC], f32, tag="acc", name="yp")
                for h in range(NH):
                    nc.tensor.matmul(yp, wt[:, h, :], aoT[c % 2][:, h, :],
                                     start=(h == 0), stop=(h == NH - 1))
                    yield
                ys = ysp.tile([P, TC], f32, tag="ys", name="ys")
                nc.vector.tensor_copy(ys, yp)
                nc.sync.dma_start(
                    out=yT_d[oi * P:(oi + 1) * P, c * TC:(c + 1) * TC],
                    in_=ys)

            return pre, gen

        pend = []          # [pre, gen] not yet prefetched
        active = []        # generators with w already fetched
        LOOKAHEAD = 4

        def refill():
            while pend and len(active) < LOOKAHEAD:
                pre, gen = pend.pop(0)
                active.append(gen(pre()))

        def pump(n):
            refill()
            while n > 0 and active:
                g = active[0]
                try:
                    next(g)
                    n -= 1
                except StopIteration:
                    active.pop(0)
                    refill()

        def pump_all():
            refill()
            while active:
                try:
                    next(active[0])
                except StopIteration:
                    active.pop(0)
                    refill()

        # ---------------- attention middle ----------------
        def relayout(c, sub, half):
            t0 = sub * 64 + 32 * half
            nc.gpsimd.tensor_copy(
                v2[sub % 2][:, 32 * half:32 * (half + 1), 0:NH],
                qkv[2][c % 2][:, :, t0:t0 + 32]
                .rearrange("p g t -> p t g"))

        state = {"gi": 0, "prev": None}

        def phase_a(c, s):
            sub = s // 8
            if s % 8 == 2 and sub + 1 < 8:
                relayout(c, sub + 1, 0)
            if s % 8 == 5 and sub + 1 < 8:
                relayout(c, sub + 1, 1)
            q_sb, k_sb = qkv[0][c % 2], qkv[1][c % 2]
            sc = psS.tile([P, 32], f32, tag="sc", name="sc")
            t0 = s * 8
            for j in range(4):
                for half in range(2):
                    t = t0 + 4 * half + j
                    nc.tensor.matmul(
                        sc[32 * j:32 * j + NH, 16 * half:16 * half + 16],
                        k_sb[:, :, t], q_sb[:, :, t],
                        start=True, stop=True,
                        tile_position=(0, 32 * j))
            es = esp.tile([P, 32], bf16, tag="es", name="es")
            nc.scalar.activation(out=es, in_=sc,
                                 func=mybir.ActivationFunctionType.Exp,
                                 scale=SC)
            gi = state["gi"]
            # one wide block-diag tile: col 140*hf + 8h + j, rows 32j+g
            bd = bds[gi % NBD]
            bdv = bd.rearrange("p (hf q) -> p hf q", hf=2)
            esv = es.rearrange("p (hf g) -> p hf g", hf=2)
            for j in range(4):
                dst = bdv[32 * j:32 * j + 16, :, j:j + 121:8]
                srcv = esv[32 * j:32 * j + 16, :, :]
                if j in (0, 3):
                    nc.gpsimd.tensor_copy(dst, srcv)
                elif j == 1:
                    nc.scalar.activation(
                        out=dst, in_=srcv,
                        func=mybir.ActivationFunctionType.Copy)
                else:
                    nc.vector.tensor_copy(dst, srcv)
            for half in range(2):
                # V block transpose: [128 d, 4t*32] -> [(32t+g), d]
                vps = psM.tile([P, P], bf16, tag="m", name="vps")
                nc.tensor.transpose(
                    vps,
                    v2[sub % 2][:, (s % 8) * 8 + 4 * half:
                                (s % 8) * 8 + 4 * half + 4, :]
                    .rearrange("p t g -> p (t g)"),
                    ident)
                vg = vgs[(2 * gi + half) % NVG]
                nc.vector.tensor_copy(vg[:, 0:HD], vps)
            state["gi"] = gi + 1
            state["prev"] = (gi, s)

        def phase_b1(c, prev):
            gi, s = prev
            av = psM.tile([P, HD + 1], f32, tag="m", name="av")
            bd = bds[gi % NBD]
            nc.tensor.matmul(av, bd[:, 0:P], vgs[(2 * gi) % NVG],
                             start=True, stop=False)
            nc.tensor.matmul(av, bd[:, 136:136 + P],
                             vgs[(2 * gi + 1) % NVG],
                             start=False, stop=True)
            invz = ivp.tile([P, 1], f32, tag="iv", name="invz")
            nc.vector.reciprocal(invz, av[:, HD:HD + 1])
            ao = aop.tile([P, P], bf16, tag="ao", name="ao")
            nc.vector.tensor_scalar_mul(ao, av[:, 0:HD], invz)
            return ao

        def phase_b2(c, prev, ao):
            gi, s = prev
            aops = psM.tile([P, P], bf16, tag="m", name="aops")
            nc.tensor.transpose(aops, ao, ident)
            nc.vector.tensor_copy(
                aoT[c % 2][:, :, 8 * s:8 * s + 8],
                aops.rearrange("p (h t) -> p h t", t=8))

        # ---------------- schedule ----------------
        load_x(0)
        load_x(1)
        for oi in range(NO):
            for m in range(3):
                pend.append(qkv_unit(0, oi, m))
        pump_all()

        for c in range(NCH):
            if c + 2 < NCH:
                load_x(c + 2)
            if c >= 1:
                for oi in range(NO):
                    pend.append(wo_unit(c - 1, oi))
            if c + 1 < NCH:
                for oi in range(NO):
                    for m in range(3):
                        pend.append(qkv_unit(c + 1, oi, m))
            relayout(c, 0, 0)
            relayout(c, 0, 1)
            state["prev"] = None
            for s in range(NG):
                prev = state["prev"]
                phase_a(c, s)
                pump(6)
                if prev is not None:
                    ao = phase_b1(c, prev)
                    pump(5)
                    phase_b2(c, prev, ao)
                else:
                    pump(5)
                pump(6)
            prev = state["prev"]
            ao = phase_b1(c, prev)
            phase_b2(c, prev, ao)
        for oi in range(NO):
            pend.append(wo_unit(NCH - 1, oi))
        pump_all()

    nc.compile()
    return nc


def _get_program():
    if "nc" not in _cached:
        _cached["nc"] = _build_program()
    return _cached["nc"]


def kernel(x, Wq, Wk, Wv, Wo):
    import ml_dtypes
    from concourse.bass_utils import run_bass_kernel_spmd

    bfd = ml_dtypes.bfloat16
    B, S, H = x.shape
    assert (B * S, H) == (NCORES * TPC, E)
    nc = _get_program()

    def pretile(W):
        # [oi, p, e, o] with row oi*128+p, col e*128+o ; W is [out, in]
        A = np.asarray(W).reshape(NO, P, NE, P).transpose(0, 3, 2, 1)
        return np.ascontiguousarray(A.reshape(E, E).astype(bfd))

    Wqp = pretile(Wq)
    Wkp = pretile(Wk)
    Wvp = pretile(Wv)
    Wop = pretile(Wo)

    xf = np.asarray(x).reshape(B * S, H)
    in_maps = []
    for i in range(NCORES):
        xT = np.ascontiguousarray(
            xf[i * TPC:(i + 1) * TPC, :].T.astype(bfd))
        in_maps.append({"xT": xT, "Wq": Wqp, "Wk": Wkp,
                       "Wv": Wvp, "Wo": Wop})

    import os
    trace = bool(int(os.environ.get("BASS_KERNEL_TRACE", "0")))
    res = run_bass_kernel_spmd(nc, in_maps, core_ids=list(range(NCORES)),
                               trace=trace)
    if trace:
        _cached["last_results"] = res
    parts = [res.results[i]["yT"].T for i in range(NCORES)]
    y = np.concatenate(parts, axis=0).reshape(B, S, H)
    return np.ascontiguousarray(y.astype(np.float32))


# revision 18
# speedup vs baseline: 2.3354x; 1.0114x over previous
"""Trainium2 Bass kernel for per-token multi-head self-attention.

Computation (per token t):
  q,k,v = x @ W{q,k,v}.T ; scores = (q_t k_t^T)/sqrt(128) over heads [16x16]
  out_t = softmax(scores) @ v_t ; y = out @ Wo.T

Sharding: data-parallel over the 16384 tokens -> 8 cores x 2048 tokens.

Fully-fused single-pass structure, all matmul operands in bf16 (fp32 PSUM
accumulation; CPU-simulated pipeline rel err ~4.4e-3 vs the 2e-2 gate):
  - One stream of "GEMM units" (QKV projection tiles and Wo output tiles)
    is interleaved ("pumped") between the small attention-middle ops so the
    PE never idles on the middle's cross-engine dependency chains.
  - qkv never round-trips through DRAM: QKV units for chunk c+1 run (as
    pump filler) during the attention middle of chunk c, writing SBUF
    double buffers.
  - Attention middle processes 8 tokens per group: per-token 16x16 score
    matmuls (4 PE column-groups x 2 rounds) -> one exp ACT -> block-diag
    [128,128] attn matrix (copies split across gpsimd/vector/scalar) ->
    one AV matmul against the PE-transposed V block with a ones column
    producing the softmax normalizer -> per-partition 1/Z scale -> one
    PE-transpose back to feature-major layout for the Wo GEMM.
Weights are host-side pre-tiled so every weight-tile DMA reads 2-4KB
contiguous runs.
"""
import math
from contextlib import ExitStack

import numpy as np

NCORES = 8
E = 2048          # hidden
NH = 16           # heads
HD = 128          # head dim
TPC = 2048        # tokens per core
TC = 512          # token chunk
P = 128
NE = E // P       # 16 contraction tiles
NO = E // P       # 16 output tiles
NCH = TPC // TC   # 4 chunks
NG = TC // 8      # 64 8-token groups per chunk
SC = 1.0 / math.sqrt(HD)

_cached = {}


_dbg = {}


def _build_program():
    import concourse.bass as bass
    import concourse.tile as tile
    from concourse import bacc, mybir
    from concourse.masks import make_identity

    f32 = mybir.dt.float32
    bf16 = mybir.dt.bfloat16

    nc = bacc.Bacc("TRN2", target_bir_lowering=False, debug=False)

    xT_d = nc.dram_tensor("xT", [E, TPC], bf16, kind="ExternalInput").ap()
    # pre-tiled weights: row oi*128+p, col e*128+o  (p = input-feature within
    # e-slice for QKV; for Wo: p = head-dim within head h, col h*128+o)
    Wq_d = nc.dram_tensor("Wq", [E, E], bf16, kind="ExternalInput").ap()
    Wk_d = nc.dram_tensor("Wk", [E, E], bf16, kind="ExternalInput").ap()
    Wv_d = nc.dram_tensor("Wv", [E, E], bf16, kind="ExternalInput").ap()
    Wo_d = nc.dram_tensor("Wo", [E, E], bf16, kind="ExternalInput").ap()
    yT_d = nc.dram_tensor("yT", [E, TPC], f32, kind="ExternalOutput").ap()

    with tile.TileContext(nc) as tc, ExitStack() as ctx:
        glob = ctx.enter_context(tc.tile_pool(name="glob", bufs=1))
        ident = glob.tile([P, P], bf16)
        make_identity(nc, ident)

        xp = ctx.enter_context(tc.tile_pool(name="xp", bufs=1))
        qkvp = ctx.enter_context(tc.tile_pool(name="qkvp", bufs=1))
        aotp = ctx.enter_context(tc.tile_pool(name="aotp", bufs=1))
        v2p = ctx.enter_context(tc.tile_pool(name="v2p", bufs=1))
        bdp = ctx.enter_context(tc.tile_pool(name="bdp", bufs=1))
        vgp = ctx.enter_context(tc.tile_pool(name="vgp", bufs=1))
        wp = ctx.enter_context(tc.tile_pool(name="wp", bufs=4))
        esp = ctx.enter_context(tc.tile_pool(name="esp", bufs=3))
        aop = ctx.enter_context(tc.tile_pool(name="aop", bufs=3))
        ivp = ctx.enter_context(tc.tile_pool(name="ivp", bufs=3))
        ysp = ctx.enter_context(tc.tile_pool(name="ysp", bufs=3))
        psG = ctx.enter_context(tc.tile_pool(name="psG", bufs=3, space="PSUM"))
        psS = ctx.enter_context(tc.tile_pool(name="psS", bufs=1, space="PSUM"))
        psM = ctx.enter_context(tc.tile_pool(name="psM", bufs=4, space="PSUM"))

        # persistent double buffers
        xb = [xp.tile([P, NE, TC], bf16, tag=f"x{i}", name=f"x{i}")
              for i in range(2)]
        qkv = [[qkvp.tile([P, NO, TC], bf16, tag=f"qkv{m}_{i}",
                          name=f"qkv{m}_{i}")
                for i in range(2)] for m in range(3)]
        aoT = [aotp.tile([P, NH, TC], bf16, tag=f"aoT{i}", name=f"aoT{i}")
               for i in range(2)]
        v2 = []
        for i in range(2):
            t = v2p.tile([P, 64, 32], bf16, tag=f"v2_{i}", name=f"v2_{i}")
            nc.vector.memset(t, 0.0)
            v2.append(t)
        NBD = 4
        bds = []
        for i in range(NBD):
            t = bdp.tile([P, 280], bf16, tag=f"bd{i}", name=f"bd{i}")
            nc.vector.memset(t, 0.0)
            bds.append(t)
        NVG = 8
        vgs = []
        for i in range(NVG):
            t = vgp.tile([P, HD + 1], bf16, tag=f"vg{i}", name=f"vg{i}")
            nc.vector.memset(t, 0.0)
            nc.vector.memset(t[:, HD:HD + 1], 1.0)
            vgs.append(t)

        wmats = [Wq_d, Wk_d, Wv_d]

        def load_x(c):
            for e in range(NE):
                nc.sync.dma_start(
                    out=xb[c % 2][:, e, :],
                    in_=xT_d[e * P:(e + 1) * P, c * TC:(c + 1) * TC])

        # ---------------- GEMM unit machinery ----------------
        # Each unit: (prefetch_fn -> returns w tile, gen_fn(w) yields per MM)
        def qkv_unit(c, oi, m):
            def pre():
                wt = wp.tile([P, NE, P], bf16, tag="w", name="w")
                wf = wt.rearrange("p e o -> p (e o)")
                nc.sync.dma_start(out=wf[:, 0:E // 2],
                                  in_=wmats[m][oi * P:(oi + 1) * P, 0:E // 2])
                nc.sync.dma_start(out=wf[:, E // 2:E],
                                  in_=wmats[m][oi * P:(oi + 1) * P, E // 2:E])
                return wt

            def gen(wt):
                acc = psG.tile([P, TC], f32, tag="acc", name="acc")
                for e in range(NE):
                    nc.tensor.matmul(acc, wt[:, e, :], xb[c % 2][:, e, :],
                                     start=(e == 0), stop=(e == NE - 1))
                    yield
                nc.scalar.activation(
                    out=qkv[m][c % 2][:, oi, :], in_=acc,
                    func=mybir.ActivationFunctionType.Copy)

            return pre, gen

        def wo_unit(c, oi):
            def pre():
                wt = wp.tile([P, NH, P], bf16, tag="w", name="w")
                wf = wt.rearrange("p h o -> p (h o)")
                nc.sync.dma_start(out=wf[:, 0:E // 2],
                                  in_=Wo_d[oi * P:(oi + 1) * P, 0:E // 2])
                nc.sync.dma_start(out=wf[:, E // 2:E],
                                  in_=Wo_d[oi * P:(oi + 1) * P, E // 2:E])
                return wt

            def gen(wt):
                yp = psG.tile([P, TC], f32, tag="acc", name="yp")
                for h in range(NH):
                    nc.tensor.matmul(yp, wt[:, h, :], aoT[c % 2][:, h, :],
                                     start=(h == 0), stop=(h == NH - 1))
                    yield
                ys = ysp.tile([P, TC], f32, tag="ys", name="ys")
                nc.vector.tensor_copy(ys, yp)
                nc.sync.dma_start(
                    out=yT_d[oi * P:(oi + 1) * P, c * TC:(c + 1) * TC],
                    in_=ys)

            return pre, gen

        pend = []          # [pre, gen] not yet prefetched
        active = []        # generators with w already fetched
        LOOKAHEAD = 4

        def refill():
            while pend and len(active) < LOOKAHEAD:
                pre, gen = pend.pop(0)
                active.append(gen(pre()))

        def pump(n):
            refill()
            while n > 0 and active:
                g = active[0]
                try:
                    next(g)
                    n -= 1
                except StopIteration:
                    active.pop(0)
                    refill()

        def pump_all():
            refill()
            while active:
                try:
                    next(active[0])
                except StopIteration:
                    active.pop(0)
                    refill()

        # ---------------- attention middle ----------------
        def relayout(c, sub, half):
            t0 = sub * 64 + 32 * half
            nc.gpsimd.tensor_copy(
                v2[sub % 2][:, 32 * half:32 * (half + 1), 0:NH],
                qkv[2][c % 2][:, :, t0:t0 + 32]
                .rearrange("p g t -> p t g"))

        state = {"gi": 0, "prev": None}

        def phase_a(c, s):
            sub = s // 8
            if s % 8 == 2 and sub + 1 < 8:
                relayout(c, sub + 1, 0)
            if s % 8 == 5 and sub + 1 < 8:
                relayout(c, sub + 1, 1)
            q_sb, k_sb = qkv[0][c % 2], qkv[1][c % 2]
            sc = psS.tile([P, 32], f32, tag="sc", name="sc")
            t0 = s * 8
            for j in range(4):
                for half in range(2):
                    t = t0 + 4 * half + j
                    nc.tensor.matmul(
                        sc[32 * j:32 * j + NH, 16 * half:16 * half + 16],
                        k_sb[:, :, t], q_sb[:, :, t],
                        start=True, stop=True,
                        tile_position=(0, 32 * j))
            es = esp.tile([P, 32], bf16, tag="es", name="es")
            nc.scalar.activation(out=es, in_=sc,
                                 func=mybir.ActivationFunctionType.Exp,
                                 scale=SC)
            gi = state["gi"]
            # one wide block-diag tile: col 140*hf + 8h + j, rows 32j+g
            bd = bds[gi % NBD]
            bdv = bd.rearrange("p (hf q) -> p hf q", hf=2)
            esv = es.rearrange("p (hf g) -> p hf g", hf=2)
            for j in range(4):
                dst = bdv[32 * j:32 * j + 16, :, j:j + 121:8]
                srcv = esv[32 * j:32 * j + 16, :, :]
                if j in (0, 3):
                    nc.gpsimd.tensor_copy(dst, srcv)
                elif j == 1:
                    nc.scalar.activation(
                        out=dst, in_=srcv,
                        func=mybir.ActivationFunctionType.Copy)
                else:
                    nc.vector.tensor_copy(dst, srcv)
            for half in range(2):
                # V block transpose: [128 d, 4t*32] -> [(32t+g), d]
                vps = psM.tile([P, P], bf16, tag="m", name="vps")
                nc.tensor.transpose(
                    vps,
                    v2[sub % 2][:, (s % 8) * 8 + 4 * half:
                                (s % 8) * 8 + 4 * half + 4, :]
                    .rearrange("p t g -> p (t g)"),
                    ident)
                vg = vgs[(2 * gi + half) % NVG]
                if half == 0:
                    nc.vector.tensor_copy(vg[:, 0:HD], vps)
                else:
                    nc.scalar.activation(
                        out=vg[:, 0:HD], in_=vps,
                        func=mybir.ActivationFunctionType.Copy)
            state["gi"] = gi + 1
            state["prev"] = (gi, s)

        def phase_b1(c, prev):
            gi, s = prev
            av = psM.tile([P, HD + 1], f32, tag="m", name="av")
            bd = bds[gi % NBD]
            nc.tensor.matmul(av, bd[:, 0:P], vgs[(2 * gi) % NVG],
                             start=True, stop=False)
            nc.tensor.matmul(av, bd[:, 136:136 + P],
                             vgs[(2 * gi + 1) % NVG],
                             start=False, stop=True)
            invz = ivp.tile([P, 1], f32, tag="iv", name="invz")
            nc.vector.reciprocal(invz, av[:, HD:HD + 1])
            ao = aop.tile([P, P], bf16, tag="ao", name="ao")
            nc.vector.tensor_scalar_mul(ao, av[:, 0:HD], invz)
            return ao

        def phase_b2(c, prev, ao):
            gi, s = prev
            aops = psM.tile([P, P], bf16, tag="m", name="aops")
            nc.tensor.transpose(aops, ao, ident)
            nc.scalar.activation(
                out=aoT[c % 2][:, :, 8 * s:8 * s + 8],
                in_=aops.rearrange("p (h t) -> p h t", t=8),
                func=mybir.ActivationFunctionType.Copy)

        # ---------------- schedule ----------------
        load_x(0)
        load_x(1)
        for oi in range(NO):
            for m in range(3):
                pend.append(qkv_unit(0, oi, m))
        pump_all()

        for c in range(NCH):
            if c + 2 < NCH:
                load_x(c + 2)
            if c >= 1:
                for oi in range(NO):
                    pend.append(wo_unit(c - 1, oi))
            if c + 1 < NCH:
                for oi in range(NO):
                    for m in range(3):
                        pend.append(qkv_unit(c + 1, oi, m))
            relayout(c, 0, 0)
            relayout(c, 0, 1)
            state["prev"] = None
            for s in range(NG):
                prev = state["prev"]
                phase_a(c, s)
                pump(6)
                if prev is not None:
                    ao = phase_b1(c, prev)
                    pump(5)
                    phase_b2(c, prev, ao)
                else:
                    pump(5)
                pump(6)
            prev = state["prev"]
            ao = phase_b1(c, prev)
            phase_b2(c, prev, ao)
        for oi in range(NO):
            pend.append(wo_unit(NCH - 1, oi))
        pump_all()

    nc.compile()
    return nc


def _get_program():
    if "nc" not in _cached:
        _cached["nc"] = _build_program()
    return _cached["nc"]


def kernel(x, Wq, Wk, Wv, Wo):
    import ml_dtypes
    from concourse.bass_utils import run_bass_kernel_spmd

    bfd = ml_dtypes.bfloat16
    B, S, H = x.shape
    assert (B * S, H) == (NCORES * TPC, E)
    nc = _get_program()

    def pretile(W):
        # [oi, p, e, o] with row oi*128+p, col e*128+o ; W is [out, in]
        A = np.asarray(W).reshape(NO, P, NE, P).transpose(0, 3, 2, 1)
        return np.ascontiguousarray(A.reshape(E, E).astype(bfd))

    Wqp = pretile(Wq)
    Wkp = pretile(Wk)
    Wvp = pretile(Wv)
    Wop = pretile(Wo)

    xf = np.asarray(x).reshape(B * S, H)
    in_maps = []
    for i in range(NCORES):
        xT = np.ascontiguousarray(
            xf[i * TPC:(i + 1) * TPC, :].T.astype(bfd))
        in_maps.append({"xT": xT, "Wq": Wqp, "Wk": Wkp,
                       "Wv": Wvp, "Wo": Wop})

    import os
    trace = bool(int(os.environ.get("BASS_KERNEL_TRACE", "0")))
    res = run_bass_kernel_spmd(nc, in_maps, core_ids=list(range(NCORES)),
                               trace=trace)
    if trace:
        _cached["last_results"] = res
    parts = [res.results[i]["yT"].T for i in range(NCORES)]
    y = np.concatenate(parts, axis=0).reshape(B, S, H)
    return np.ascontiguousarray(y.astype(np.float32))
